# revision 29
# baseline (speedup 1.0000x reference)
"""AttentiveGRU2 Trainium2 Bass kernel.

Model (see reference):
  edge-softmax over incoming edges per dst node, attention-weighted
  gather of projected node features, segment-sum per dst, ELU, GRUCell.

Strategy (8 NeuronCores, SPMD, no collectives) — v3:
  * Host sorts edges by dst window (128 consecutive node ids); 392 windows
    are bin-packed (sorted by A-edge count, dealt 8-per-position) onto
    49 positions x 8 cores so each position's slot count is the max over
    only 8 windows instead of all 392 (~10% fewer padded slots).
  * Softmax shift-invariance: a_e = exp(l_e)/sum exp(l_e); the division by
    the segment denominator is folded through the segment sum:
    c_v = W @ (sum_e ex_e nf[src_e]) / (sum_e ex_e) + b.
  * The gather is latency-bound (~10 ns/row on one SWDGE queue).  It is
    split across all 4 SWDGE queues (ucode max) which hardware-parallelizes
    the descriptor streams (~5x), with gbufs=3 gather-tile rings so the
    descriptor generation for batch b+2 never stalls on batch b's
    consumers (keeps the queues' duty cycle high).
  * dma_gather needs int16 indices but V=50000 > 32767, so the nf table is
    addressed through two overlapping row views: A = rows [0, 32768)
    (src < 32768) and B = rows [17232, 50000) (idx = src - 17232).
    A rides queues {0,1}, B rides {2,3}.  Pad slots gather row 0 and are
    killed by dst_local = -1 in the one-hot.
  * Everything on-chip is bf16 (table, one-hot, weights, nf^T): DVE runs
    at 2x, PE matmuls get FWL weight loads, SBUF footprint halves.
    PSUM accumulation stays fp32 (rel err ~9e-3 < 2e-2).
  * Two-pass structure per iteration: pass 1 does gathers + one-hot builds
    + Gs=G*ex + per-window PE accumulation psum_ud += O.T @ [Gs|ex], then
    ctx = u/den straight into an SBUF ctx_all tile (so the PE never stalls
    mid-edge-stream on node-phase work).  Pass 2 (groups of GW=2 windows):
    PE transpose of ctx, cT = W_proj @ ctx^T, feature-major GRU so all
    biases are per-partition (folded into Act bias operands; zero bias
    matmuls), ELU's exp(x)-1 rewritten as 2t/(1-t) with t=tanh(x/2) so the
    whole node phase stays in the 'sigmoid_and_others' activation table
    (zero 1.3us table reloads), blend, relu, DMA out feature-major.
  * Output is [128, nodes] feature-major per core; host transposes and
    applies the inverse window permutation.
"""

import numpy as np

V, E, F = 50000, 800000, 128
NC = 8
WPC = 49              # windows per core
NPC = WPC * 128       # 6272 node slots per core
WTOT = NC * WPC       # 392 windows total
WPB = 2               # windows per gather batch
S_SPLIT = 32768       # src < S -> table A
OFF_B = V - 32768     # 17232; table B rows [OFF_B, V)

_compiled = {}


def _build_nc(T_win, sA=None, sB=None, skip_gather=False, skip_onehot=False,
              skip_mm=False, skip_node=False, repeat=1, one_act=False,
              n_q=1, sp=False, bf16_tab=False, n_calls=None):
    import concourse.bass as bass
    import concourse.bacc as bacc
    import concourse.mybir as mybir
    import concourse.tile as tile

    f32 = mybir.dt.float32
    bf16 = mybir.dt.bfloat16
    gdt = bf16 if bf16_tab else f32
    i16 = mybir.dt.int16
    AF = mybir.ActivationFunctionType
    OP = mybir.AluOpType
    AF_E = AF.Sigmoid if one_act else AF.Exp
    AF_T = AF.Sigmoid if one_act else AF.Tanh
    AF_R = AF.Sigmoid if one_act else AF.Relu

    if sA is None:
        sA, sB = T_win, 0   # legacy path unused
    SW = sA + sB            # slots per window
    T = WPC * SW            # tile-columns per core
    LA = WPC * sA * 128     # A-gather idx count per core
    LB = WPC * sB * 128

    nc = bacc.Bacc("TRN2", target_bir_lowering=False, debug=False,
                   num_devices=NC, num_swdge_queues=max(1, n_q))

    # ---- DRAM parameters ----
    idxa_d = nc.dram_tensor("idxa", [128, LA // 16], i16,
                            kind="ExternalInput")
    idxb_d = nc.dram_tensor("idxb", [128, LB // 16], i16,
                            kind="ExternalInput")
    dstla_d = nc.dram_tensor("dstla", [128, WPC * sA], f32,
                             kind="ExternalInput")
    dstlb_d = nc.dram_tensor("dstlb", [128, WPC * sB], f32,
                             kind="ExternalInput")
    logita_d = nc.dram_tensor("logita", [128, WPC * sA], f32,
                              kind="ExternalInput")
    logitb_d = nc.dram_tensor("logitb", [128, WPC * sB], f32,
                              kind="ExternalInput")
    table_d = nc.dram_tensor("table", [V, F], gdt, kind="ExternalInput")
    nfT_d = nc.dram_tensor("nfT", [128, NPC], f32, kind="ExternalInput")
    wprojT_d = nc.dram_tensor("wprojT", [128, 128], f32, kind="ExternalInput")
    wihT_d = nc.dram_tensor("wihT", [128, 384], f32, kind="ExternalInput")
    whhT_d = nc.dram_tensor("whhT", [128, 384], f32, kind="ExternalInput")
    bproj_d = nc.dram_tensor("bproj", [1, 128], f32, kind="ExternalInput")
    brz_d = nc.dram_tensor("brz", [1, 256], f32, kind="ExternalInput")
    bni_d = nc.dram_tensor("bni", [1, 128], f32, kind="ExternalInput")
    bnh_d = nc.dram_tensor("bnh", [1, 128], f32, kind="ExternalInput")
    iota_d = nc.dram_tensor("iota", [128, 128], f32, kind="ExternalInput")
    ident_d = nc.dram_tensor("ident", [128, 128], f32, kind="ExternalInput")
    onesc_d = nc.dram_tensor("onesc", [128, 1], f32, kind="ExternalInput")
    onesr_d = nc.dram_tensor("onesr", [1, 128], f32, kind="ExternalInput")
    tableb_d = nc.dram_tensor("tableb", [32768, 128], gdt,
                              kind="ExternalInput")
    out_d = nc.dram_tensor("out", [NPC, 128], f32, kind="ExternalOutput")

    tabA = table_d[0:32768, :]
    tabB = tableb_d[:]

    with tile.TileContext(nc) as tc:
        with (
            tc.tile_pool(name="const", bufs=1) as cpool,
            tc.tile_pool(name="gat", bufs=2) as gpool,
            tc.tile_pool(name="oh", bufs=2) as opool,
            tc.tile_pool(name="wrk", bufs=2) as wpool,
            tc.tile_pool(name="pedge", bufs=1, space="PSUM") as pe_pool,
            tc.tile_pool(name="pnode", bufs=1, space="PSUM") as pn_pool,
        ):
            def load(pool, name, dram, shape, dtype=f32):
                t = pool.tile(shape, dtype, tag=name)
                nc.sync.dma_start(t[:], dram[:])
                return t

            iota_sb = load(cpool, "iota", iota_d, [128, 128])
            ident_sb = load(cpool, "ident", ident_d, [128, 128])
            onesc_sb = load(cpool, "onesc", onesc_d, [128, 1])
            onesr_sb = load(cpool, "onesr", onesr_d, [1, 128])
            wproj_sb = load(cpool, "wproj", wprojT_d, [128, 128])
            wih_sb = load(cpool, "wih", wihT_d, [128, 384])
            whh_sb = load(cpool, "whh", whhT_d, [128, 384])
            bproj_sb = load(cpool, "bproj", bproj_d, [1, 128])
            brz_sb = load(cpool, "brz", brz_d, [1, 256])
            bni_sb = load(cpool, "bni", bni_d, [1, 128])
            bnh_sb = load(cpool, "bnh", bnh_d, [1, 128])
            idxa_sb = load(cpool, "idxa", idxa_d, [128, LA // 16], i16)
            idxb_sb = load(cpool, "idxb", idxb_d, [128, LB // 16], i16)
            dstla_sb = load(cpool, "dstla", dstla_d, [128, WPC * sA])
            dstlb_sb = load(cpool, "dstlb", dstlb_d, [128, WPC * sB])
            nfT_sb = load(cpool, "nfT", nfT_d, [128, NPC])

            exa_sb = cpool.tile([128, WPC * sA], f32, tag="exa")
            nc.sync.dma_start(exa_sb[:], logita_d[:])
            nc.scalar.activation(exa_sb[:], exa_sb[:], AF.Exp)
            exb_sb = cpool.tile([128, WPC * sB], f32, tag="exb")
            nc.sync.dma_start(exb_sb[:], logitb_d[:])
            nc.scalar.activation(exb_sb[:], exb_sb[:], AF.Exp)

            def apx(base, dims):
                return bass.AP(base.tensor, base.offset,
                               [list(base.ap[0])] + dims)

            n_batches = (WPC + WPB - 1) // WPB
            GA_static = GB_static = None
            if skip_gather:
                GA_static = cpool.tile([128, WPB * sA, 128], gdt, tag="GAs")
                nc.gpsimd.memset(GA_static[:], 0.0)
                GB_static = cpool.tile([128, WPB * sB, 128], gdt, tag="GBs")
                nc.gpsimd.memset(GB_static[:], 0.0)


            for _rep in range(repeat):
              for b in range(n_batches):
                w0 = b * WPB
                nw = min(WPB, WPC - w0)
                if skip_gather:
                    GA, GB = GA_static, GB_static
                else:
                    GA = gpool.tile([128, WPB * sA, 128], gdt, tag="GA")
                    GB = gpool.tile([128, WPB * sB, 128], gdt, tag="GB")

                    ncall = n_calls if n_calls else n_q

                    def qgather(G, tab, idx_sb, t0, nt):
                        # split [t0, t0+nt) tiles across ncall calls on n_q qs
                        per = (nt + ncall - 1) // ncall
                        q = 0
                        o = 0
                        while o < nt:
                            cn = min(per, nt - o)
                            ni = cn * 128
                            nc.gpsimd.dma_gather(
                                out_ap=G[:, o:o + cn, :],
                                in_ap=tab,
                                idxs_ap=idx_sb[:, ((t0 + o) * 128) // 16:
                                               ((t0 + o + cn) * 128) // 16],
                                num_idxs=ni, num_idxs_reg=ni, elem_size=128,
                                single_packet=sp, queue_num=q,
                            )
                            q = (q + 1) % max(1, n_q)
                            o += cn

                    qgather(GA, tabA, idxa_sb, w0 * sA, nw * sA)
                    qgather(GB, tabB, idxb_sb, w0 * sB, nw * sB)
                ntA, ntB = nw * sA, nw * sB
                cA0, cB0 = w0 * sA, w0 * sB
                OA = opool.tile([128, WPB * sA, 128], f32, tag="OA")
                OB = opool.tile([128, WPB * sB, 128], f32, tag="OB")
                GsA = gpool.tile([128, WPB * sA, 132], f32, tag="GsA")
                GsB = gpool.tile([128, WPB * sB, 132], f32, tag="GsB")
                if not skip_onehot:
                    for (O, dstl_sb, nt, c0) in (
                            (OA, dstla_sb, ntA, cA0),
                            (OB, dstlb_sb, ntB, cB0)):
                        nc.vector.tensor_tensor(
                            out=O[:, 0:nt, :],
                            in0=apx(iota_sb[:], [[0, nt], [1, 128]]),
                            in1=apx(dstl_sb[:, c0:c0 + nt],
                                    [[1, nt], [0, 128]]),
                            op=OP.is_equal)
                for (G, Gs, ex_sb, nt, c0) in (
                        (GA, GsA, exa_sb, ntA, cA0),
                        (GB, GsB, exb_sb, ntB, cB0)):
                    nc.vector.tensor_tensor(
                        out=Gs[:, 0:nt, 0:128], in0=G[:, 0:nt, :],
                        in1=apx(ex_sb[:, c0:c0 + nt], [[1, nt], [0, 128]]),
                        op=OP.mult)
                    nc.vector.tensor_copy(out=Gs[:, 0:nt, 128:129],
                                          in_=ex_sb[:, c0:c0 + nt])
                for wl in range(nw):
                    w = w0 + wl
                    psum_ud = pe_pool.tile([128, 132], f32, tag="psum_ud",
                                           bufs=2)
                    if not skip_mm:
                        for s_ in range(SW):
                            if s_ < sA:
                                Olh = OA[:, wl * sA + s_, :]
                                Grh = GsA[:, wl * sA + s_, 0:129]
                            else:
                                Olh = OB[:, wl * sB + (s_ - sA), :]
                                Grh = GsB[:, wl * sB + (s_ - sA), 0:129]
                            nc.tensor.matmul(
                                psum_ud[:, 0:129], lhsT=Olh, rhs=Grh,
                                start=(s_ == 0), stop=(s_ == SW - 1),
                            )

                    if skip_node:
                        continue
                    # ---- node phase for window w ----
                    den = wpool.tile([128, 1], f32, tag="den")
                    nc.vector.tensor_scalar(
                        out=den[:], in0=psum_ud[:, 128:129], scalar1=1e-30,
                        scalar2=None, op0=OP.max)
                    rec = wpool.tile([128, 1], f32, tag="rec")
                    nc.vector.reciprocal(rec[:], den[:])
                    ctx_t = wpool.tile([128, 128], f32, tag="ctx_t")
                    nc.vector.tensor_scalar(
                        out=ctx_t[:], in0=psum_ud[:, 0:128],
                        scalar1=rec[:, 0:1],
                        scalar2=None, op0=OP.mult)

                    ptr = pn_pool.tile([128, 128], f32, tag="ptr", bufs=2)
                    nc.tensor.transpose(ptr[:], ctx_t[:], ident_sb[:])
                    ctxT = wpool.tile([128, 128], f32, tag="ctxT")
                    nc.vector.tensor_copy(out=ctxT[:], in_=ptr[:])

                    # cT = W_proj @ ctx~.T + b_proj  (H on partitions)
                    psum_cT = pn_pool.tile([128, 128], f32, tag="psum_cT",
                                           bufs=2)
                    nc.tensor.matmul(psum_cT[:], lhsT=wproj_sb[:],
                                     rhs=ctxT[:], start=True, stop=False)
                    nc.tensor.matmul(psum_cT[:], lhsT=bproj_sb[:],
                                     rhs=onesr_sb[:], start=False, stop=True)

                    # elu(cT) = max(cT,0) + exp(min(cT,0)) - 1
                    cmin = wpool.tile([128, 128], f32, tag="cmin")
                    nc.vector.tensor_scalar(out=cmin[:], in0=psum_cT[:],
                                            scalar1=0.0, scalar2=None,
                                            op0=OP.min)
                    cexp = wpool.tile([128, 128], f32, tag="cexp")
                    nc.scalar.activation(cexp[:], cmin[:], AF_E)
                    crelu = wpool.tile([128, 128], f32, tag="crelu")
                    nc.vector.tensor_scalar(out=crelu[:], in0=psum_cT[:],
                                            scalar1=0.0, scalar2=None,
                                            op0=OP.max)
                    ce1 = wpool.tile([128, 128], f32, tag="ce1")
                    nc.vector.tensor_scalar(out=ce1[:], in0=cexp[:],
                                            scalar1=1.0, scalar2=None,
                                            op0=OP.subtract)
                    ctxT2 = wpool.tile([128, 128], f32, tag="ctxT2")
                    nc.vector.tensor_tensor(out=ctxT2[:], in0=ce1[:],
                                            in1=crelu[:], op=OP.add)

                    nfT_tile = nfT_sb[:, w * 128:(w + 1) * 128]
                    # gates PSUM: [0:256]=r|z (gi+gh), [256:384]=i_n,
                    # [384:512]=h_n
                    psum_g = pn_pool.tile([128, 512], f32, tag="psum_g",
                                          bufs=2)
                    psum_rz = psum_g[:, 0:256]
                    nc.tensor.matmul(psum_rz, lhsT=ctxT2[:],
                                     rhs=wih_sb[:, 0:256],
                                     start=True, stop=False)
                    nc.tensor.matmul(psum_rz, lhsT=nfT_tile,
                                     rhs=whh_sb[:, 0:256],
                                     start=False, stop=False)
                    nc.tensor.matmul(psum_rz, lhsT=onesr_sb[:],
                                     rhs=brz_sb[:], start=False, stop=True)
                    psum_nh = psum_g[:, 256:512]
                    nc.tensor.matmul(psum_nh[:, 0:128], lhsT=ctxT2[:],
                                     rhs=wih_sb[:, 256:384],
                                     start=True, stop=False)
                    nc.tensor.matmul(psum_nh[:, 0:128], lhsT=onesr_sb[:],
                                     rhs=bni_sb[:], start=False, stop=True)
                    nc.tensor.matmul(psum_nh[:, 128:256], lhsT=nfT_tile,
                                     rhs=whh_sb[:, 256:384],
                                     start=True, stop=False)
                    nc.tensor.matmul(psum_nh[:, 128:256], lhsT=onesr_sb[:],
                                     rhs=bnh_sb[:], start=False, stop=True)

                    rzs = wpool.tile([128, 256], f32, tag="rzs")
                    nc.scalar.activation(rzs[:], psum_rz, AF.Sigmoid)
                    nt1 = wpool.tile([128, 128], f32, tag="nt1")
                    nc.vector.tensor_tensor(out=nt1[:], in0=rzs[:, 0:128],
                                            in1=psum_nh[:, 128:256],
                                            op=OP.mult)
                    nt2 = wpool.tile([128, 128], f32, tag="nt2")
                    nc.vector.tensor_tensor(out=nt2[:], in0=nt1[:],
                                            in1=psum_nh[:, 0:128],
                                            op=OP.add)
                    nn = wpool.tile([128, 128], f32, tag="nn")
                    nc.scalar.activation(nn[:], nt2[:], AF_T)

                    pnf = pn_pool.tile([128, 128], f32, tag="ptr", bufs=2)
                    nc.tensor.transpose(pnf[:], nfT_tile, ident_sb[:])
                    df = wpool.tile([128, 128], f32, tag="df")
                    nc.vector.tensor_tensor(out=df[:], in0=pnf[:], in1=nn[:],
                                            op=OP.subtract)
                    dz = wpool.tile([128, 128], f32, tag="dz")
                    nc.vector.tensor_tensor(out=dz[:], in0=df[:],
                                            in1=rzs[:, 128:256], op=OP.mult)
                    hh = wpool.tile([128, 128], f32, tag="hh")
                    nc.vector.tensor_tensor(out=hh[:], in0=dz[:], in1=nn[:],
                                            op=OP.add)
                    outt = wpool.tile([128, 128], f32, tag="outt")
                    nc.scalar.activation(outt[:], hh[:], AF_R)
                    nc.sync.dma_start(out_d[w * 128:(w + 1) * 128, :],
                                      outt[:])

    nc.compile()
    return nc


def _build_v2(sA, sB, repeat=1, n_q=4, WPBv=4, GW=2, skip_gather=False,
              skip_onehot=False, skip_mm=False, skip_node=False):
    """bf16 edge phase + feature-major node phase, 4-queue gathers."""
    import concourse.bass as bass
    import concourse.bacc as bacc
    import concourse.mybir as mybir
    import concourse.tile as tile

    f32 = mybir.dt.float32
    bf16 = mybir.dt.bfloat16
    i16 = mybir.dt.int16
    AF = mybir.ActivationFunctionType
    OP = mybir.AluOpType

    SW = sA + sB
    LA = WPC * sA * 128
    LB = WPC * sB * 128

    nc = bacc.Bacc("TRN2", target_bir_lowering=False, debug=False,
                   num_devices=NC, num_swdge_queues=max(1, n_q))

    idxa_d = nc.dram_tensor("idxa", [128, LA // 16], i16,
                            kind="ExternalInput")
    idxb_d = nc.dram_tensor("idxb", [128, LB // 16], i16,
                            kind="ExternalInput")
    dstla_d = nc.dram_tensor("dstla", [128, WPC * sA], bf16,
                             kind="ExternalInput")
    dstlb_d = nc.dram_tensor("dstlb", [128, WPC * sB], bf16,
                             kind="ExternalInput")
    logita_d = nc.dram_tensor("logita", [128, WPC * sA], f32,
                              kind="ExternalInput")
    logitb_d = nc.dram_tensor("logitb", [128, WPC * sB], f32,
                              kind="ExternalInput")
    table_d = nc.dram_tensor("table", [V, F], bf16, kind="ExternalInput")
    tableb_d = nc.dram_tensor("tableb", [32768, 128], bf16,
                              kind="ExternalInput")
    nfT_d = nc.dram_tensor("nfT", [128, NPC], bf16, kind="ExternalInput")
    wprojT_d = nc.dram_tensor("wprojT", [128, 128], bf16,
                              kind="ExternalInput")
    wihT_d = nc.dram_tensor("wihT", [128, 384], bf16, kind="ExternalInput")
    whhT_d = nc.dram_tensor("whhT", [128, 384], bf16, kind="ExternalInput")
    ident_d = nc.dram_tensor("ident", [128, 128], bf16, kind="ExternalInput")
    iota_d = nc.dram_tensor("iota", [128, 128], bf16, kind="ExternalInput")
    bcols_d = nc.dram_tensor("bcols", [128, 5], f32, kind="ExternalInput")
    out_d = nc.dram_tensor("out", [128, NPC], f32, kind="ExternalOutput")

    tabA = table_d[0:32768, :]
    tabB = tableb_d[:]

    with tile.TileContext(nc) as tc:
        with (
            tc.tile_pool(name="const", bufs=1) as cpool,
            tc.tile_pool(name="gat", bufs=2) as gpool,
            tc.tile_pool(name="oh", bufs=2) as opool,
            tc.tile_pool(name="wrk", bufs=2) as wpool,
            tc.tile_pool(name="pedge", bufs=1, space="PSUM") as pe_pool,
            tc.tile_pool(name="pnode", bufs=1, space="PSUM") as pn_pool,
        ):
            def load(pool, name, dram, shape, dtype=f32):
                t = pool.tile(shape, dtype, tag=name)
                nc.sync.dma_start(t[:], dram[:])
                return t

            iota_sb = load(cpool, "iota", iota_d, [128, 128], bf16)
            ident_sb = load(cpool, "ident", ident_d, [128, 128], bf16)
            wproj_sb = load(cpool, "wproj", wprojT_d, [128, 128], bf16)
            wih_sb = load(cpool, "wih", wihT_d, [128, 384], bf16)
            whh_sb = load(cpool, "whh", whhT_d, [128, 384], bf16)
            bcols_sb = load(cpool, "bcols", bcols_d, [128, 5], f32)
            bproj_c = bcols_sb[:, 0:1]
            br_c = bcols_sb[:, 1:2]
            bz_c = bcols_sb[:, 2:3]
            bni_c = bcols_sb[:, 3:4]
            bnh_c = bcols_sb[:, 4:5]
            idxa_sb = load(cpool, "idxa", idxa_d, [128, LA // 16], i16)
            idxb_sb = load(cpool, "idxb", idxb_d, [128, LB // 16], i16)
            dstla_sb = load(cpool, "dstla", dstla_d, [128, WPC * sA], bf16)
            dstlb_sb = load(cpool, "dstlb", dstlb_d, [128, WPC * sB], bf16)
            nfT_sb = load(cpool, "nfT", nfT_d, [128, NPC], bf16)

            lstage = cpool.tile([128, WPC * sA], f32, tag="lstage")
            exa_sb = cpool.tile([128, WPC * sA], bf16, tag="exa")
            nc.sync.dma_start(lstage[:], logita_d[:])
            nc.scalar.activation(exa_sb[:], lstage[:], AF.Exp)
            lstageb = cpool.tile([128, WPC * sB], f32, tag="lstageb")
            exb_sb = cpool.tile([128, WPC * sB], bf16, tag="exb")
            nc.sync.dma_start(lstageb[:], logitb_d[:])
            nc.scalar.activation(exb_sb[:], lstageb[:], AF.Exp)

            def apx(base, dims):
                return bass.AP(base.tensor, base.offset,
                               [list(base.ap[0])] + dims)

            n_batches = (WPC + WPBv - 1) // WPBv
            GA_static = GB_static = None
            if skip_gather:
                GA_static = cpool.tile([128, WPBv * sA, 128], bf16,
                                       tag="GAs")
                nc.gpsimd.memset(GA_static[:], 0.0)
                GB_static = cpool.tile([128, WPBv * sB, 128], bf16,
                                       tag="GBs")
                nc.gpsimd.memset(GB_static[:], 0.0)

            for _rep in range(repeat):
              for b in range(n_batches):
                w0 = b * WPBv
                nw = min(WPBv, WPC - w0)
                if skip_gather:
                    GA, GB = GA_static, GB_static
                else:
                    GA = gpool.tile([128, WPBv * sA, 128], bf16, tag="GA")
                    GB = gpool.tile([128, WPBv * sB, 128], bf16, tag="GB")

                    def qgather(G, tab, idx_sb, t0, nt, q0):
                        half = (nt + 1) // 2
                        for i, (o, cn) in enumerate(
                                ((0, half), (half, nt - half))):
                            if cn <= 0:
                                continue
                            ni = cn * 128
                            nc.gpsimd.dma_gather(
                                out_ap=G[:, o:o + cn, :],
                                in_ap=tab,
                                idxs_ap=idx_sb[:, ((t0 + o) * 128) // 16:
                                               ((t0 + o + cn) * 128) // 16],
                                num_idxs=ni, num_idxs_reg=ni, elem_size=128,
                                single_packet=False,
                                queue_num=(q0 + i) % max(1, n_q),
                            )

                    qgather(GA, tabA, idxa_sb, w0 * sA, nw * sA, 0)
                    qgather(GB, tabB, idxb_sb, w0 * sB, nw * sB,
                            2 % max(1, n_q))
                ntA, ntB = nw * sA, nw * sB
                cA0, cB0 = w0 * sA, w0 * sB
                OA = opool.tile([128, WPBv * sA, 128], bf16, tag="OA")
                OB = opool.tile([128, WPBv * sB, 128], bf16, tag="OB")
                GsA = gpool.tile([128, WPBv * sA, 132], bf16, tag="GsA")
                GsB = gpool.tile([128, WPBv * sB, 132], bf16, tag="GsB")
                if not skip_onehot:
                    for (O, dstl_sb, nt, c0) in (
                            (OA, dstla_sb, ntA, cA0),
                            (OB, dstlb_sb, ntB, cB0)):
                        nc.vector.tensor_tensor(
                            out=O[:, 0:nt, :],
                            in0=apx(iota_sb[:], [[0, nt], [1, 128]]),
                            in1=apx(dstl_sb[:, c0:c0 + nt],
                                    [[1, nt], [0, 128]]),
                            op=OP.is_equal)
                for (G, Gs, ex_sb, nt, c0) in (
                        (GA, GsA, exa_sb, ntA, cA0),
                        (GB, GsB, exb_sb, ntB, cB0)):
                    nc.vector.tensor_tensor(
                        out=Gs[:, 0:nt, 0:128], in0=G[:, 0:nt, :],
                        in1=apx(ex_sb[:, c0:c0 + nt], [[1, nt], [0, 128]]),
                        op=OP.mult)
                    nc.vector.tensor_copy(out=Gs[:, 0:nt, 128:129],
                                          in_=ex_sb[:, c0:c0 + nt])

                for g0 in range(0, nw, GW):
                    ng = min(GW, nw - g0)
                    gn = ng * 128
                    ctxTg = wpool.tile([128, GW * 128], bf16, tag="ctxTg")
                    for wl in range(g0, g0 + ng):
                        w = w0 + wl
                        psum_ud = pe_pool.tile([128, 132], f32,
                                               tag="psum_ud", bufs=2)
                        if not skip_mm:
                            for s_ in range(SW):
                                if s_ < sA:
                                    Olh = OA[:, wl * sA + s_, :]
                                    Grh = GsA[:, wl * sA + s_, 0:129]
                                else:
                                    Olh = OB[:, wl * sB + (s_ - sA), :]
                                    Grh = GsB[:, wl * sB + (s_ - sA), 0:129]
                                nc.tensor.matmul(
                                    psum_ud[:, 0:129], lhsT=Olh, rhs=Grh,
                                    start=(s_ == 0), stop=(s_ == SW - 1),
                                )
                        if skip_node:
                            continue
                        # ---- per-window: ctx = u/den, transpose ----
                        den = wpool.tile([128, 1], f32, tag="den")
                        nc.vector.tensor_scalar(
                            out=den[:], in0=psum_ud[:, 128:129],
                            scalar1=1e-30, scalar2=None, op0=OP.max)
                        rec = wpool.tile([128, 1], f32, tag="rec")
                        nc.vector.reciprocal(rec[:], den[:])
                        ctx_t = wpool.tile([128, 128], bf16, tag="ctx_t")
                        nc.vector.tensor_scalar(
                            out=ctx_t[:], in0=psum_ud[:, 0:128],
                            scalar1=rec[:, 0:1], scalar2=None, op0=OP.mult)
                        ptr = pn_pool.tile([128, 128], bf16, tag="ptr",
                                           bufs=1)
                        nc.tensor.transpose(ptr[:], ctx_t[:], ident_sb[:])
                        nc.vector.tensor_copy(
                            out=ctxTg[:, (wl - g0) * 128:(wl - g0 + 1) * 128],
                            in_=ptr[:])

                    if skip_node:
                        continue
                    # ---- group node phase (f-major) ----
                    psum_cT = pn_pool.tile([128, GW * 128], f32,
                                           tag="psum_cT", bufs=1)
                    nc.tensor.matmul(psum_cT[:, 0:gn], lhsT=wproj_sb[:],
                                     rhs=ctxTg[:, 0:gn], start=True,
                                     stop=True)
                    # ELU(x+bproj) via tanh: e^x-1 = 2t/(1-t), t=tanh(x/2)
                    cmin = wpool.tile([128, GW * 128], f32, tag="cmin")
                    nc.vector.tensor_scalar(
                        out=cmin[:, 0:gn], in0=psum_cT[:, 0:gn],
                        scalar1=bproj_c, scalar2=0.0, op0=OP.add,
                        op1=OP.min)
                    th = wpool.tile([128, GW * 128], f32, tag="th")
                    nc.scalar.activation(th[:, 0:gn], cmin[:, 0:gn],
                                         AF.Tanh, scale=0.5)
                    omt = wpool.tile([128, GW * 128], f32, tag="omt")
                    nc.vector.tensor_scalar(
                        out=omt[:, 0:gn], in0=th[:, 0:gn], scalar1=-1.0,
                        scalar2=1.0, op0=OP.mult, op1=OP.add)
                    rv = wpool.tile([128, GW * 128], f32, tag="rv")
                    nc.vector.reciprocal(rv[:, 0:gn], omt[:, 0:gn])
                    eneg = wpool.tile([128, GW * 128], f32, tag="eneg")
                    nc.vector.scalar_tensor_tensor(
                        out=eneg[:, 0:gn], in0=th[:, 0:gn], scalar=2.0,
                        in1=rv[:, 0:gn], op0=OP.mult, op1=OP.mult)
                    crelu = wpool.tile([128, GW * 128], f32, tag="crelu")
                    nc.vector.tensor_scalar(
                        out=crelu[:, 0:gn], in0=psum_cT[:, 0:gn],
                        scalar1=bproj_c, scalar2=0.0, op0=OP.add,
                        op1=OP.max)
                    ctx2 = wpool.tile([128, GW * 128], bf16, tag="ctx2")
                    nc.vector.tensor_tensor(
                        out=ctx2[:, 0:gn], in0=eneg[:, 0:gn],
                        in1=crelu[:, 0:gn], op=OP.add)

                    nfTg = nfT_sb[:, (w0 + g0) * 128:(w0 + g0 + ng) * 128]
                    psum_g = pn_pool.tile([128, GW * 512], f32,
                                          tag="psum_g", bufs=2)
                    psum_rz = psum_g[:, 0:GW * 256]
                    psum_nh = psum_g[:, GW * 256:GW * 512]
                    GWn = GW * 128
                    nc.tensor.matmul(psum_rz[:, 0:gn],
                                     lhsT=wih_sb[:, 0:128],
                                     rhs=ctx2[:, 0:gn],
                                     start=True, stop=False)
                    nc.tensor.matmul(psum_rz[:, 0:gn],
                                     lhsT=whh_sb[:, 0:128], rhs=nfTg,
                                     start=False, stop=True)
                    nc.tensor.matmul(psum_rz[:, GWn:GWn + gn],
                                     lhsT=wih_sb[:, 128:256],
                                     rhs=ctx2[:, 0:gn],
                                     start=True, stop=False)
                    nc.tensor.matmul(psum_rz[:, GWn:GWn + gn],
                                     lhsT=whh_sb[:, 128:256], rhs=nfTg,
                                     start=False, stop=True)
                    nc.tensor.matmul(psum_nh[:, 0:gn],
                                     lhsT=wih_sb[:, 256:384],
                                     rhs=ctx2[:, 0:gn],
                                     start=True, stop=True)
                    nc.tensor.matmul(psum_nh[:, GWn:GWn + gn],
                                     lhsT=whh_sb[:, 256:384], rhs=nfTg,
                                     start=True, stop=True)

                    sig_r = wpool.tile([128, GW * 128], f32, tag="sig_r")
                    nc.scalar.activation(sig_r[:, 0:gn], psum_rz[:, 0:gn],
                                         AF.Sigmoid, bias=br_c)
                    sig_z = wpool.tile([128, GW * 128], bf16, tag="sig_z")
                    nc.scalar.activation(sig_z[:, 0:gn],
                                         psum_rz[:, GWn:GWn + gn],
                                         AF.Sigmoid, bias=bz_c)
                    hnr = wpool.tile([128, GW * 128], f32, tag="hnr")
                    nc.vector.scalar_tensor_tensor(
                        out=hnr[:, 0:gn], in0=psum_nh[:, GWn:GWn + gn],
                        scalar=bnh_c, in1=sig_r[:, 0:gn],
                        op0=OP.add, op1=OP.mult)
                    npre = wpool.tile([128, GW * 128], f32, tag="npre")
                    nc.vector.tensor_tensor(
                        out=npre[:, 0:gn], in0=hnr[:, 0:gn],
                        in1=psum_nh[:, 0:gn], op=OP.add)
                    nn = wpool.tile([128, GW * 128], bf16, tag="nn")
                    nc.scalar.activation(nn[:, 0:gn], npre[:, 0:gn],
                                         AF.Tanh, bias=bni_c)
                    df = wpool.tile([128, GW * 128], bf16, tag="df")
                    nc.vector.tensor_tensor(
                        out=df[:, 0:gn], in0=nfTg, in1=nn[:, 0:gn],
                        op=OP.subtract)
                    dz = wpool.tile([128, GW * 128], bf16, tag="dz")
                    nc.vector.tensor_tensor(
                        out=dz[:, 0:gn], in0=df[:, 0:gn],
                        in1=sig_z[:, 0:gn], op=OP.mult)
                    hh = wpool.tile([128, GW * 128], bf16, tag="hh")
                    nc.vector.tensor_tensor(
                        out=hh[:, 0:gn], in0=dz[:, 0:gn],
                        in1=nn[:, 0:gn], op=OP.add)
                    outg = wpool.tile([128, GW * 128], f32, tag="outg")
                    nc.vector.tensor_scalar(
                        out=outg[:, 0:gn], in0=hh[:, 0:gn], scalar1=0.0,
                        scalar2=None, op0=OP.max)
                    nc.sync.dma_start(
                        out_d[:, (w0 + g0) * 128:(w0 + g0 + ng) * 128],
                        outg[:, 0:gn])

    nc.compile()
    return nc


def _prep(edge_logits, node_feats, W_proj, b_proj, w_ih, w_hh, b_ih, b_hh,
          src, dst, bf16_tab=False):
    """Host-side sharding. Returns (T_win, sA, sB, in_maps)."""
    logits = np.asarray(edge_logits, np.float32).reshape(-1)
    src = np.asarray(src, np.int64)
    dst = np.asarray(dst, np.int64)

    is_b = (src >= S_SPLIT).astype(np.int64)
    win = dst // 128
    key = win * 2 + is_b
    order = np.argsort(key, kind="stable")
    key_s = key[order]
    src_s = src[order]
    dst_s = dst[order]
    log_s = logits[order]

    counts = np.bincount(key_s, minlength=WTOT * 2)
    cA = counts[0::2]
    cB = counts[1::2]
    sA = int((cA.max() + 127) // 128)
    sB = int((cB.max() + 127) // 128)
    T_win = sA + sB

    starts = np.zeros(WTOT * 2, np.int64)
    starts[1:] = np.cumsum(counts)[:-1]
    pos = np.arange(E, dtype=np.int64) - starts[key_s]

    # flat slot index within the core-ordered [WTOT, sA*128 | sB*128] arrays
    winv = key_s // 2
    grp = key_s % 2
    idxA = np.zeros(WTOT * sA * 128, np.int16)
    idxB = np.zeros(WTOT * sB * 128, np.int16)
    dstlA = np.full(WTOT * sA * 128, -1.0, np.float32)
    dstlB = np.full(WTOT * sB * 128, -1.0, np.float32)
    logA = np.zeros(WTOT * sA * 128, np.float32)
    logB = np.zeros(WTOT * sB * 128, np.float32)

    mA = grp == 0
    mB = ~mA
    flatA = winv[mA] * (sA * 128) + pos[mA]
    flatB = winv[mB] * (sB * 128) + pos[mB]
    idxA[flatA] = src_s[mA].astype(np.int16)
    idxB[flatB] = (src_s[mB] - OFF_B).astype(np.int16)
    dstlA[flatA] = (dst_s[mA] - winv[mA] * 128).astype(np.float32)
    dstlB[flatB] = (dst_s[mB] - winv[mB] * 128).astype(np.float32)
    logA[flatA] = log_s[mA]
    logB[flatB] = log_s[mB]

    def core_tiles(a, slots):
        a = a.reshape(WTOT, slots, 128)
        return [np.ascontiguousarray(
            a[k * WPC:(k + 1) * WPC].transpose(2, 0, 1)
            .reshape(128, WPC * slots)) for k in range(NC)]

    dstlA_cores = core_tiles(dstlA, sA)
    dstlB_cores = core_tiles(dstlB, sB)
    logA_cores = core_tiles(logA, sA)
    logB_cores = core_tiles(logB, sB)

    def core_idx(a, slots):
        a = a.reshape(WTOT, slots * 128)
        out = []
        for k in range(NC):
            flat = a[k * WPC:(k + 1) * WPC].reshape(-1)
            blk = flat.reshape(-1, 16).T      # [16, L/16], i -> [i%16,i//16]
            out.append(np.ascontiguousarray(np.tile(blk, (8, 1))))
        return out

    idxA_cores = core_idx(idxA, sA)
    idxB_cores = core_idx(idxB, sB)

    nf = np.asarray(node_feats, np.float32)
    nf_pad = np.zeros((NC * NPC, F), np.float32)
    nf_pad[:V] = nf

    if bf16_tab:
        import ml_dtypes
        table = np.ascontiguousarray(nf.astype(ml_dtypes.bfloat16))
        tableb = np.ascontiguousarray(table[OFF_B:])
    else:
        table = np.ascontiguousarray(nf)
        tableb = np.ascontiguousarray(nf[OFF_B:])
    wprojT = np.ascontiguousarray(np.asarray(W_proj, np.float32).T)
    wihT = np.ascontiguousarray(np.asarray(w_ih, np.float32).T)
    whhT = np.ascontiguousarray(np.asarray(w_hh, np.float32).T)
    bproj = np.asarray(b_proj, np.float32).reshape(1, 128)
    bih = np.asarray(b_ih, np.float32).reshape(384)
    bhh = np.asarray(b_hh, np.float32).reshape(384)
    brz = (bih[0:256] + bhh[0:256]).reshape(1, 256)
    bni = bih[256:384].reshape(1, 128)
    bnh = bhh[256:384].reshape(1, 128)
    iota = np.tile(np.arange(128, dtype=np.float32), (128, 1))
    ident = np.eye(128, dtype=np.float32)
    onesc = np.ones((128, 1), np.float32)
    onesr = np.ones((1, 128), np.float32)

    in_maps = []
    for k in range(NC):
        sl = nf_pad[k * NPC:(k + 1) * NPC]
        nfT = np.ascontiguousarray(sl.T)
        in_maps.append({
            "idxa": idxA_cores[k], "idxb": idxB_cores[k],
            "dstla": dstlA_cores[k], "dstlb": dstlB_cores[k],
            "logita": logA_cores[k], "logitb": logB_cores[k],
            "table": table, "tableb": tableb,
            "nfT": nfT,
            "wprojT": wprojT, "wihT": wihT, "whhT": whhT,
            "bproj": bproj, "brz": brz, "bni": bni, "bnh": bnh,
            "iota": iota, "ident": ident,
            "onesc": onesc, "onesr": onesr,
        })
    return T_win, sA, sB, in_maps


def _build_v3(sAl, sBl, repeat=1, n_q=4, WPBv=4, GW=2, skip_gather=False,
              skip_onehot=False, skip_mm=False, skip_node=False,
              probe=None, gbufs=2, qmode="ab", streami=False):
    """Two-pass: edge phase stores ctx to SBUF; node pass runs after.

    sAl/sBl: per-position slot counts (len WPC) from host bin-packing.
    """
    import concourse.bass as bass
    import concourse.bacc as bacc
    import concourse.mybir as mybir
    import concourse.tile as tile

    f32 = mybir.dt.float32
    bf16 = mybir.dt.bfloat16
    i16 = mybir.dt.int16
    AF = mybir.ActivationFunctionType
    OP = mybir.AluOpType

    if probe == "gather":
        skip_onehot = skip_mm = skip_node = True
    elif probe == "edge":
        skip_mm = skip_node = True
    offA = [0]
    offB = [0]
    for j in range(WPC):
        offA.append(offA[-1] + sAl[j])
        offB.append(offB[-1] + sBl[j])
    ncA, ncB = offA[-1], offB[-1]
    LA, LB = ncA * 128, ncB * 128
    n_batches = (WPC + WPBv - 1) // WPBv
    maxbA = max(offA[min(b * WPBv + WPBv, WPC)] - offA[b * WPBv]
                for b in range(n_batches))
    maxbB = max(offB[min(b * WPBv + WPBv, WPC)] - offB[b * WPBv]
                for b in range(n_batches))

    nc = bacc.Bacc("TRN2", target_bir_lowering=False, debug=False,
                   num_devices=NC, num_swdge_queues=max(1, n_q))

    idxa_d = nc.dram_tensor("idxa", [128, LA // 16], i16,
                            kind="ExternalInput")
    idxb_d = nc.dram_tensor("idxb", [128, LB // 16], i16,
                            kind="ExternalInput")
    dstla_d = nc.dram_tensor("dstla", [128, ncA], bf16,
                             kind="ExternalInput")
    dstlb_d = nc.dram_tensor("dstlb", [128, ncB], bf16,
                             kind="ExternalInput")
    logita_d = nc.dram_tensor("logita", [128, ncA], f32,
                              kind="ExternalInput")
    logitb_d = nc.dram_tensor("logitb", [128, ncB], f32,
                              kind="ExternalInput")
    table_d = nc.dram_tensor("table", [V, F], bf16, kind="ExternalInput")
    tableb_d = nc.dram_tensor("tableb", [32768, 128], bf16,
                              kind="ExternalInput")
    nfT_d = nc.dram_tensor("nfT", [128, NPC], bf16, kind="ExternalInput")
    wprojT_d = nc.dram_tensor("wprojT", [128, 128], bf16,
                              kind="ExternalInput")
    wihT_d = nc.dram_tensor("wihT", [128, 384], bf16, kind="ExternalInput")
    whhT_d = nc.dram_tensor("whhT", [128, 384], bf16, kind="ExternalInput")
    ident_d = nc.dram_tensor("ident", [128, 128], bf16, kind="ExternalInput")
    iota_d = nc.dram_tensor("iota", [128, 128], bf16, kind="ExternalInput")
    bcols_d = nc.dram_tensor("bcols", [128, 5], f32, kind="ExternalInput")
    out_d = nc.dram_tensor("out", [128, NPC], f32, kind="ExternalOutput")

    tabA = table_d[0:32768, :]
    tabB = tableb_d[:]

    with tile.TileContext(nc) as tc:
        with (
            tc.tile_pool(name="const", bufs=1) as cpool,
            tc.tile_pool(name="ctxp", bufs=2) as xpool,
            tc.tile_pool(name="gat", bufs=2) as gpool,
            tc.tile_pool(name="oh", bufs=2) as opool,
            tc.tile_pool(name="wrk", bufs=2) as wpool,
            tc.tile_pool(name="pedge", bufs=1, space="PSUM") as pe_pool,
            tc.tile_pool(name="pnode", bufs=1, space="PSUM") as pn_pool,
        ):
            def load(pool, name, dram, shape, dtype=f32):
                t = pool.tile(shape, dtype, tag=name)
                nc.sync.dma_start(t[:], dram[:])
                return t

            iota_sb = load(cpool, "iota", iota_d, [128, 128], bf16)
            ident_sb = load(cpool, "ident", ident_d, [128, 128], bf16)
            wproj_sb = load(cpool, "wproj", wprojT_d, [128, 128], bf16)
            wih_sb = load(cpool, "wih", wihT_d, [128, 384], bf16)
            whh_sb = load(cpool, "whh", whhT_d, [128, 384], bf16)
            bcols_sb = load(cpool, "bcols", bcols_d, [128, 5], f32)
            bproj_c = bcols_sb[:, 0:1]
            br_c = bcols_sb[:, 1:2]
            bz_c = bcols_sb[:, 2:3]
            bni_c = bcols_sb[:, 3:4]
            bnh_c = bcols_sb[:, 4:5]
            if not streami:
                idxa_sb = load(cpool, "idxa", idxa_d, [128, LA // 16], i16)
                idxb_sb = load(cpool, "idxb", idxb_d, [128, LB // 16], i16)
            dstla_sb = load(cpool, "dstla", dstla_d, [128, ncA], bf16)
            dstlb_sb = load(cpool, "dstlb", dstlb_d, [128, ncB], bf16)
            nfT_sb = load(cpool, "nfT", nfT_d, [128, NPC], bf16)

            lstage = cpool.tile([128, ncA], f32, tag="lstage")
            exa_sb = cpool.tile([128, ncA], bf16, tag="exa")
            nc.sync.dma_start(lstage[:], logita_d[:])
            nc.scalar.activation(exa_sb[:], lstage[:], AF.Exp)
            lstageb = cpool.tile([128, ncB], f32, tag="lstageb")
            exb_sb = cpool.tile([128, ncB], bf16, tag="exb")
            nc.sync.dma_start(lstageb[:], logitb_d[:])
            nc.scalar.activation(exb_sb[:], lstageb[:], AF.Exp)

            def apx(base, dims):
                return bass.AP(base.tensor, base.offset,
                               [list(base.ap[0])] + dims)

            GA_static = GB_static = None
            if skip_gather:
                GA_static = cpool.tile([128, maxbA, 128], bf16, tag="GAs")
                nc.gpsimd.memset(GA_static[:], 0.0)
                GB_static = cpool.tile([128, maxbB, 128], bf16, tag="GBs")
                nc.gpsimd.memset(GB_static[:], 0.0)

            for _rep in range(repeat):
              ctx_all = xpool.tile([128, WPC * 128], bf16, tag="ctx_all")
              # ---- pass 1: gather + edge matmuls + ctx ----
              for b in range(n_batches):
                w0 = b * WPBv
                nw = min(WPBv, WPC - w0)
                a0, a1 = offA[w0], offA[w0 + nw]
                b0, b1 = offB[w0], offB[w0 + nw]
                bA, bB = a1 - a0, b1 - b0
                if skip_gather:
                    GA, GB = GA_static, GB_static
                else:
                    GA = gpool.tile([128, maxbA, 128], bf16, tag="GA",
                                    bufs=gbufs)
                    GB = gpool.tile([128, maxbB, 128], bf16, tag="GB",
                                    bufs=gbufs)
                    if streami:
                        ia_t = gpool.tile([128, maxbA * 8], i16,
                                          tag="ia_t", bufs=gbufs)
                        nc.sync.dma_start(ia_t[:, 0:bA * 8],
                                          idxa_d[:, a0 * 8:a1 * 8])
                        ib_t = gpool.tile([128, maxbB * 8], i16,
                                          tag="ib_t", bufs=gbufs)
                        nc.sync.dma_start(ib_t[:, 0:bB * 8],
                                          idxb_d[:, b0 * 8:b1 * 8])
                        ia_sb, ib_sb = ia_t, ib_t
                        ta0, tb0 = 0, 0
                    else:
                        ia_sb, ib_sb = idxa_sb, idxb_sb
                        ta0, tb0 = a0, b0

                    def qgather(G, tab, idx_sb, t0, nt, qlist, sizes=None):
                        np_ = len(qlist)
                        if sizes is None:
                            per = (nt + np_ - 1) // np_
                            sizes = [per] * np_
                        o = 0
                        for i in range(np_):
                            cn = min(sizes[i], nt - o)
                            if cn <= 0:
                                break
                            ni = cn * 128
                            nc.gpsimd.dma_gather(
                                out_ap=G[:, o:o + cn, :],
                                in_ap=tab,
                                idxs_ap=idx_sb[:, ((t0 + o) * 128) // 16:
                                               ((t0 + o + cn) * 128) // 16],
                                num_idxs=ni, num_idxs_reg=ni, elem_size=128,
                                single_packet=False,
                                queue_num=qlist[i] % max(1, n_q),
                            )
                            o += cn

                    sa_sz = sb_sz = None
                    if qmode == "swap" and (b % 2) == 1:
                        qa, qb_ = (2, 3), (0, 1)
                    elif qmode == "quad":
                        qa, qb_ = (0, 1, 2, 3), (3, 2, 1, 0)
                    elif qmode == "bal":
                        # equalize all 4 queues: each carries (bA+bB)/4
                        qa, qb_ = (0, 1, 2), (2, 3)
                        q_tot = (bA + bB + 3) // 4
                        sa_sz = [q_tot, q_tot, max(0, bA - 2 * q_tot)]
                        sb_sz = [max(0, bB - q_tot), q_tot]
                    else:
                        qa, qb_ = (0, 1), (2, 3)
                    qgather(GA, tabA, ia_sb, ta0, bA, qa, sa_sz)
                    qgather(GB, tabB, ib_sb, tb0, bB, qb_, sb_sz)
                OA = opool.tile([128, maxbA, 128], bf16, tag="OA")
                OB = opool.tile([128, maxbB, 128], bf16, tag="OB")
                GsA = gpool.tile([128, maxbA, 132], bf16, tag="GsA")
                GsB = gpool.tile([128, maxbB, 132], bf16, tag="GsB")
                if not skip_onehot:
                    for (O, dstl_sb, nt, c0) in (
                            (OA, dstla_sb, bA, a0),
                            (OB, dstlb_sb, bB, b0)):
                        nc.vector.tensor_tensor(
                            out=O[:, 0:nt, :],
                            in0=apx(iota_sb[:], [[0, nt], [1, 128]]),
                            in1=apx(dstl_sb[:, c0:c0 + nt],
                                    [[1, nt], [0, 128]]),
                            op=OP.is_equal)
                if probe != "gather":
                    for (G, Gs, ex_sb, nt, c0) in (
                            (GA, GsA, exa_sb, bA, a0),
                            (GB, GsB, exb_sb, bB, b0)):
                        nc.vector.tensor_tensor(
                            out=Gs[:, 0:nt, 0:128], in0=G[:, 0:nt, :],
                            in1=apx(ex_sb[:, c0:c0 + nt],
                                    [[1, nt], [0, 128]]),
                            op=OP.mult)
                        nc.vector.tensor_copy(out=Gs[:, 0:nt, 128:129],
                                              in_=ex_sb[:, c0:c0 + nt])

                if probe is not None:
                    # anti-DCE: give every gather/build a live consumer
                    pr = wpool.tile([128, 4], f32, tag="probe")
                    if probe == "gather":
                        srcs = [GA[:, 0, 0:1], GA[:, (bA + 1) // 2, 0:1],
                                GB[:, 0, 0:1], GB[:, (bB + 1) // 2, 0:1]]
                    else:
                        srcs = [GsA[:, 0, 0:1], GsB[:, 0, 0:1],
                                OA[:, 0, 0:1], OB[:, 0, 0:1]]
                    for i, s in enumerate(srcs):
                        nc.vector.tensor_copy(out=pr[:, i:i + 1], in_=s)
                    nc.sync.dma_start(out_d[:, b * 4:b * 4 + 4], pr[:])
                    continue

                for wl in range(nw):
                    j = w0 + wl
                    la = offA[j] - a0
                    lb = offB[j] - b0
                    sAj, sBj = sAl[j], sBl[j]
                    SWj = sAj + sBj
                    psum_ud = pe_pool.tile([128, 132], f32,
                                           tag="psum_ud", bufs=2)
                    if not skip_mm:
                        for s_ in range(SWj):
                            if s_ < sAj:
                                Olh = OA[:, la + s_, :]
                                Grh = GsA[:, la + s_, 0:129]
                            else:
                                Olh = OB[:, lb + (s_ - sAj), :]
                                Grh = GsB[:, lb + (s_ - sAj), 0:129]
                            nc.tensor.matmul(
                                psum_ud[:, 0:129], lhsT=Olh, rhs=Grh,
                                start=(s_ == 0), stop=(s_ == SWj - 1),
                            )
                    den = wpool.tile([128, 1], f32, tag="den")
                    nc.vector.tensor_scalar(
                        out=den[:], in0=psum_ud[:, 128:129],
                        scalar1=1e-30, scalar2=None, op0=OP.max)
                    rec = wpool.tile([128, 1], f32, tag="rec")
                    nc.vector.reciprocal(rec[:], den[:])
                    nc.vector.tensor_scalar(
                        out=ctx_all[:, j * 128:(j + 1) * 128],
                        in0=psum_ud[:, 0:128],
                        scalar1=rec[:, 0:1], scalar2=None, op0=OP.mult)

              # ---- pass 2: node phase over groups of GW positions ----
              if skip_node:
                  continue
              for g0 in range(0, WPC, GW):
                    ng = min(GW, WPC - g0)
                    gn = ng * 128
                    GWn = GW * 128
                    ctxTg = wpool.tile([128, GW * 128], bf16, tag="ctxTg")
                    for wl in range(ng):
                        ptr = pn_pool.tile([128, 128], bf16, tag="ptr",
                                           bufs=1)
                        nc.tensor.transpose(
                            ptr[:],
                            ctx_all[:, (g0 + wl) * 128:(g0 + wl + 1) * 128],
                            ident_sb[:])
                        nc.vector.tensor_copy(
                            out=ctxTg[:, wl * 128:(wl + 1) * 128],
                            in_=ptr[:])

                    psum_cT = pn_pool.tile([128, GW * 128], f32,
                                           tag="psum_cT", bufs=1)
                    nc.tensor.matmul(psum_cT[:, 0:gn], lhsT=wproj_sb[:],
                                     rhs=ctxTg[:, 0:gn], start=True,
                                     stop=True)
                    cmin = wpool.tile([128, GW * 128], f32, tag="cmin")
                    nc.vector.tensor_scalar(
                        out=cmin[:, 0:gn], in0=psum_cT[:, 0:gn],
                        scalar1=bproj_c, scalar2=0.0, op0=OP.add,
                        op1=OP.min)
                    th = wpool.tile([128, GW * 128], f32, tag="th")
                    nc.scalar.activation(th[:, 0:gn], cmin[:, 0:gn],
                                         AF.Tanh, scale=0.5)
                    omt = wpool.tile([128, GW * 128], f32, tag="omt")
                    nc.vector.tensor_scalar(
                        out=omt[:, 0:gn], in0=th[:, 0:gn], scalar1=-1.0,
                        scalar2=1.0, op0=OP.mult, op1=OP.add)
                    rv = wpool.tile([128, GW * 128], f32, tag="rv")
                    nc.vector.reciprocal(rv[:, 0:gn], omt[:, 0:gn])
                    eneg = wpool.tile([128, GW * 128], f32, tag="eneg")
                    nc.vector.scalar_tensor_tensor(
                        out=eneg[:, 0:gn], in0=th[:, 0:gn], scalar=2.0,
                        in1=rv[:, 0:gn], op0=OP.mult, op1=OP.mult)
                    crelu = wpool.tile([128, GW * 128], f32, tag="crelu")
                    nc.vector.tensor_scalar(
                        out=crelu[:, 0:gn], in0=psum_cT[:, 0:gn],
                        scalar1=bproj_c, scalar2=0.0, op0=OP.add,
                        op1=OP.max)
                    ctx2 = wpool.tile([128, GW * 128], bf16, tag="ctx2")
                    nc.vector.tensor_tensor(
                        out=ctx2[:, 0:gn], in0=eneg[:, 0:gn],
                        in1=crelu[:, 0:gn], op=OP.add)

                    nfTg = nfT_sb[:, g0 * 128:(g0 + ng) * 128]
                    gb = 2 if GW <= 2 else 1
                    psum_rz = pn_pool.tile([128, GW * 256], f32,
                                           tag="psum_rz", bufs=gb)
                    psum_nh = pn_pool.tile([128, GW * 256], f32,
                                           tag="psum_nh", bufs=gb)
                    nc.tensor.matmul(psum_rz[:, 0:gn],
                                     lhsT=wih_sb[:, 0:128],
                                     rhs=ctx2[:, 0:gn],
                                     start=True, stop=False)
                    nc.tensor.matmul(psum_rz[:, 0:gn],
                                     lhsT=whh_sb[:, 0:128], rhs=nfTg,
                                     start=False, stop=True)
                    nc.tensor.matmul(psum_rz[:, GWn:GWn + gn],
                                     lhsT=wih_sb[:, 128:256],
                                     rhs=ctx2[:, 0:gn],
                                     start=True, stop=False)
                    nc.tensor.matmul(psum_rz[:, GWn:GWn + gn],
                                     lhsT=whh_sb[:, 128:256], rhs=nfTg,
                                     start=False, stop=True)
                    nc.tensor.matmul(psum_nh[:, 0:gn],
                                     lhsT=wih_sb[:, 256:384],
                                     rhs=ctx2[:, 0:gn],
                                     start=True, stop=True)
                    nc.tensor.matmul(psum_nh[:, GWn:GWn + gn],
                                     lhsT=whh_sb[:, 256:384], rhs=nfTg,
                                     start=True, stop=True)

                    sig_r = wpool.tile([128, GW * 128], f32, tag="sig_r")
                    nc.scalar.activation(sig_r[:, 0:gn], psum_rz[:, 0:gn],
                                         AF.Sigmoid, bias=br_c)
                    sig_z = wpool.tile([128, GW * 128], bf16, tag="sig_z")
                    nc.scalar.activation(sig_z[:, 0:gn],
                                         psum_rz[:, GWn:GWn + gn],
                                         AF.Sigmoid, bias=bz_c)
                    hnr = wpool.tile([128, GW * 128], f32, tag="hnr")
                    nc.vector.scalar_tensor_tensor(
                        out=hnr[:, 0:gn], in0=psum_nh[:, GWn:GWn + gn],
                        scalar=bnh_c, in1=sig_r[:, 0:gn],
                        op0=OP.add, op1=OP.mult)
                    npre = wpool.tile([128, GW * 128], f32, tag="npre")
                    nc.vector.tensor_tensor(
                        out=npre[:, 0:gn], in0=hnr[:, 0:gn],
                        in1=psum_nh[:, 0:gn], op=OP.add)
                    nn = wpool.tile([128, GW * 128], bf16, tag="nn")
                    nc.scalar.activation(nn[:, 0:gn], npre[:, 0:gn],
                                         AF.Tanh, bias=bni_c)
                    df = wpool.tile([128, GW * 128], bf16, tag="df")
                    nc.vector.tensor_tensor(
                        out=df[:, 0:gn], in0=nfTg, in1=nn[:, 0:gn],
                        op=OP.subtract)
                    dz = wpool.tile([128, GW * 128], bf16, tag="dz")
                    nc.vector.tensor_tensor(
                        out=dz[:, 0:gn], in0=df[:, 0:gn],
                        in1=sig_z[:, 0:gn], op=OP.mult)
                    hh = wpool.tile([128, GW * 128], bf16, tag="hh")
                    nc.vector.tensor_tensor(
                        out=hh[:, 0:gn], in0=dz[:, 0:gn],
                        in1=nn[:, 0:gn], op=OP.add)
                    outg = wpool.tile([128, GW * 128], f32, tag="outg")
                    nc.vector.tensor_scalar(
                        out=outg[:, 0:gn], in0=hh[:, 0:gn], scalar1=0.0,
                        scalar2=None, op0=OP.max)
                    nc.sync.dma_start(
                        out_d[:, g0 * 128:(g0 + ng) * 128],
                        outg[:, 0:gn])

    nc.compile()
    return nc


def _prep_v3(edge_logits, node_feats, W_proj, b_proj, w_ih, w_hh, b_ih,
             b_hh, src, dst):
    """Host prep for v3: bin-packed window->position assignment."""
    import ml_dtypes
    bf = ml_dtypes.bfloat16
    logits = np.asarray(edge_logits, np.float32).reshape(-1)
    src = np.asarray(src, np.int64)
    dst = np.asarray(dst, np.int64)

    win = dst // 128
    is_b = (src >= S_SPLIT).astype(np.int64)
    cA = np.bincount(win[is_b == 0], minlength=WTOT)
    cB = np.bincount(win[is_b == 1], minlength=WTOT)

    order = np.argsort(cA, kind="stable")[::-1]   # rank -> window
    rank = np.empty(WTOT, np.int64)
    rank[order] = np.arange(WTOT)
    # position j, core k <- window order[j*NC + k]
    posw = rank // NC      # window -> position
    corew = rank % NC      # window -> core
    wA = cA[order].reshape(WPC, NC)
    wB = cB[order].reshape(WPC, NC)
    sAl = tuple(int(x) for x in np.ceil(wA.max(1) / 128).astype(int))
    sBl = tuple(int(x) for x in np.ceil(wB.max(1) / 128).astype(int))
    offA = np.zeros(WPC + 1, np.int64)
    offA[1:] = np.cumsum(sAl)
    offB = np.zeros(WPC + 1, np.int64)
    offB[1:] = np.cumsum(sBl)
    LAc = int(offA[-1]) * 128
    LBc = int(offB[-1]) * 128

    key = win * 2 + is_b
    order_e = np.argsort(key, kind="stable")
    key_s = key[order_e]
    src_s = src[order_e]
    dst_s = dst[order_e]
    log_s = logits[order_e]
    counts = np.bincount(key_s, minlength=WTOT * 2)
    starts = np.zeros(WTOT * 2, np.int64)
    starts[1:] = np.cumsum(counts)[:-1]
    pos = np.arange(E, dtype=np.int64) - starts[key_s]

    winv = key_s // 2
    grp = key_s % 2
    kv = corew[winv]
    jv = posw[winv]

    idxA = np.zeros(NC * LAc, np.int16)
    idxB = np.zeros(NC * LBc, np.int16)
    dstlA = np.full(NC * LAc, -1.0, np.float32)
    dstlB = np.full(NC * LBc, -1.0, np.float32)
    logA = np.zeros(NC * LAc, np.float32)
    logB = np.zeros(NC * LBc, np.float32)

    mA = grp == 0
    mB = ~mA
    flatA = kv[mA] * LAc + offA[jv[mA]] * 128 + pos[mA]
    flatB = kv[mB] * LBc + offB[jv[mB]] * 128 + pos[mB]
    idxA[flatA] = src_s[mA].astype(np.int16)
    idxB[flatB] = (src_s[mB] - OFF_B).astype(np.int16)
    dstlA[flatA] = (dst_s[mA] - winv[mA] * 128).astype(np.float32)
    dstlB[flatB] = (dst_s[mB] - winv[mB] * 128).astype(np.float32)
    logA[flatA] = log_s[mA]
    logB[flatB] = log_s[mB]

    def core_tiles(a, L, dt):
        a = a.reshape(NC, L // 128, 128)
        return [np.ascontiguousarray(a[k].T.astype(dt)) for k in range(NC)]

    dstlA_cores = core_tiles(dstlA, LAc, bf)
    dstlB_cores = core_tiles(dstlB, LBc, bf)
    logA_cores = core_tiles(logA, LAc, np.float32)
    logB_cores = core_tiles(logB, LBc, np.float32)

    def core_idx(a, L):
        a = a.reshape(NC, L)
        out = []
        for k in range(NC):
            blk = a[k].reshape(-1, 16).T
            out.append(np.ascontiguousarray(np.tile(blk, (8, 1))))
        return out

    idxA_cores = core_idx(idxA, LAc)
    idxB_cores = core_idx(idxB, LBc)

    nf = np.asarray(node_feats, np.float32)
    nf_pad = np.zeros((WTOT * 128, F), np.float32)
    nf_pad[:V] = nf

    table = np.ascontiguousarray(nf.astype(bf))
    tableb = np.ascontiguousarray(table[OFF_B:])
    wprojT = np.ascontiguousarray(np.asarray(W_proj, np.float32).T.astype(bf))
    wihT = np.ascontiguousarray(np.asarray(w_ih, np.float32).T.astype(bf))
    whhT = np.ascontiguousarray(np.asarray(w_hh, np.float32).T.astype(bf))
    bih = np.asarray(b_ih, np.float32).reshape(384)
    bhh = np.asarray(b_hh, np.float32).reshape(384)
    bcols = np.ascontiguousarray(np.stack([
        np.asarray(b_proj, np.float32).reshape(128),
        (bih + bhh)[0:128], (bih + bhh)[128:256],
        bih[256:384], bhh[256:384]], axis=1))
    iota = np.tile(np.arange(128, dtype=np.float32), (128, 1)).astype(bf)
    ident = np.eye(128, dtype=np.float32).astype(bf)

    ow = order.reshape(WPC, NC)   # [position, core] -> window
    in_maps = []
    for k in range(NC):
        widx = ow[:, k]
        node_ids = (widx[:, None] * 128 +
                    np.arange(128)[None, :]).reshape(-1)
        nfT = np.ascontiguousarray(nf_pad[node_ids].T.astype(bf))
        in_maps.append({
            "idxa": idxA_cores[k], "idxb": idxB_cores[k],
            "dstla": dstlA_cores[k], "dstlb": dstlB_cores[k],
            "logita": logA_cores[k], "logitb": logB_cores[k],
            "table": table, "tableb": tableb,
            "nfT": nfT,
            "wprojT": wprojT, "wihT": wihT, "whhT": whhT,
            "bcols": bcols, "iota": iota, "ident": ident,
        })
    return sAl, sBl, order, in_maps


def _unshard_v3(results, order):
    """results: list of per-core 'out' arrays [128, NPC]."""
    big = np.stack(results)                       # [NC, 128, NPC]
    big = big.reshape(NC, 128, WPC, 128)
    # [WPC(pos), NC(core), 128(node), 128(feat)] ordered by rank
    by_rank = big.transpose(2, 0, 3, 1).reshape(WTOT, 128, F)
    out = np.empty((WTOT, 128, F), np.float32)
    out[order.reshape(WPC, NC).reshape(-1)] = by_rank
    return out.reshape(WTOT * 128, F)[:V]


def _prep_v2(edge_logits, node_feats, W_proj, b_proj, w_ih, w_hh, b_ih,
             b_hh, src, dst):
    """Host-side sharding for the v2 kernel (bf16, f-major out)."""
    import ml_dtypes
    bf = ml_dtypes.bfloat16
    logits = np.asarray(edge_logits, np.float32).reshape(-1)
    src = np.asarray(src, np.int64)
    dst = np.asarray(dst, np.int64)

    is_b = (src >= S_SPLIT).astype(np.int64)
    win = dst // 128
    key = win * 2 + is_b
    order = np.argsort(key, kind="stable")
    key_s = key[order]
    src_s = src[order]
    dst_s = dst[order]
    log_s = logits[order]

    counts = np.bincount(key_s, minlength=WTOT * 2)
    cA = counts[0::2]
    cB = counts[1::2]
    sA = int((cA.max() + 127) // 128)
    sB = int((cB.max() + 127) // 128)

    starts = np.zeros(WTOT * 2, np.int64)
    starts[1:] = np.cumsum(counts)[:-1]
    pos = np.arange(E, dtype=np.int64) - starts[key_s]

    winv = key_s // 2
    grp = key_s % 2
    idxA = np.zeros(WTOT * sA * 128, np.int16)
    idxB = np.zeros(WTOT * sB * 128, np.int16)
    dstlA = np.full(WTOT * sA * 128, -1.0, np.float32)
    dstlB = np.full(WTOT * sB * 128, -1.0, np.float32)
    logA = np.zeros(WTOT * sA * 128, np.float32)
    logB = np.zeros(WTOT * sB * 128, np.float32)

    mA = grp == 0
    mB = ~mA
    flatA = winv[mA] * (sA * 128) + pos[mA]
    flatB = winv[mB] * (sB * 128) + pos[mB]
    idxA[flatA] = src_s[mA].astype(np.int16)
    idxB[flatB] = (src_s[mB] - OFF_B).astype(np.int16)
    dstlA[flatA] = (dst_s[mA] - winv[mA] * 128).astype(np.float32)
    dstlB[flatB] = (dst_s[mB] - winv[mB] * 128).astype(np.float32)
    logA[flatA] = log_s[mA]
    logB[flatB] = log_s[mB]

    def core_tiles(a, slots, dt):
        a = a.reshape(WTOT, slots, 128)
        return [np.ascontiguousarray(
            a[k * WPC:(k + 1) * WPC].transpose(2, 0, 1)
            .reshape(128, WPC * slots).astype(dt)) for k in range(NC)]

    dstlA_cores = core_tiles(dstlA, sA, bf)
    dstlB_cores = core_tiles(dstlB, sB, bf)
    logA_cores = core_tiles(logA, sA, np.float32)
    logB_cores = core_tiles(logB, sB, np.float32)

    def core_idx(a, slots):
        a = a.reshape(WTOT, slots * 128)
        out = []
        for k in range(NC):
            flat = a[k * WPC:(k + 1) * WPC].reshape(-1)
            blk = flat.reshape(-1, 16).T
            out.append(np.ascontiguousarray(np.tile(blk, (8, 1))))
        return out

    idxA_cores = core_idx(idxA, sA)
    idxB_cores = core_idx(idxB, sB)

    nf = np.asarray(node_feats, np.float32)
    nf_pad = np.zeros((NC * NPC, F), np.float32)
    nf_pad[:V] = nf

    table = np.ascontiguousarray(nf.astype(bf))
    tableb = np.ascontiguousarray(table[OFF_B:])
    wprojT = np.ascontiguousarray(np.asarray(W_proj, np.float32).T.astype(bf))
    wihT = np.ascontiguousarray(np.asarray(w_ih, np.float32).T.astype(bf))
    whhT = np.ascontiguousarray(np.asarray(w_hh, np.float32).T.astype(bf))
    bih = np.asarray(b_ih, np.float32).reshape(384)
    bhh = np.asarray(b_hh, np.float32).reshape(384)
    bcols = np.stack([
        np.asarray(b_proj, np.float32).reshape(128),
        (bih + bhh)[0:128], (bih + bhh)[128:256],
        bih[256:384], bhh[256:384]], axis=1)
    bcols = np.ascontiguousarray(bcols)
    iota = np.tile(np.arange(128, dtype=np.float32), (128, 1)).astype(bf)
    ident = np.eye(128, dtype=np.float32).astype(bf)

    in_maps = []
    for k in range(NC):
        sl = nf_pad[k * NPC:(k + 1) * NPC]
        nfT = np.ascontiguousarray(sl.T.astype(bf))
        in_maps.append({
            "idxa": idxA_cores[k], "idxb": idxB_cores[k],
            "dstla": dstlA_cores[k], "dstlb": dstlB_cores[k],
            "logita": logA_cores[k], "logitb": logB_cores[k],
            "table": table, "tableb": tableb,
            "nfT": nfT,
            "wprojT": wprojT, "wihT": wihT, "whhT": whhT,
            "bcols": bcols, "iota": iota, "ident": ident,
        })
    return sA, sB, in_maps


def kernel(edge_logits, node_feats, W_proj, b_proj, w_ih, w_hh, b_ih, b_hh,
           src, dst):
    from concourse.bass_utils import run_bass_kernel_spmd

    sAl, sBl, order, in_maps = _prep_v3(edge_logits, node_feats, W_proj,
                                        b_proj, w_ih, w_hh, b_ih, b_hh,
                                        src, dst)
    key = ("v3", sAl, sBl)
    if key not in _compiled:
        _compiled[key] = _build_v3(sAl, sBl, WPBv=3, gbufs=3)
    nc = _compiled[key]

    res = run_bass_kernel_spmd(nc, in_maps, list(range(NC)))
    out = _unshard_v3([res.results[k]["out"] for k in range(NC)], order)
    return np.ascontiguousarray(out).astype(np.float32)



# revision 32
# speedup vs baseline: 2.7770x; 2.7770x over previous
"""AttentiveGRU2 Trainium2 Bass kernel.

Model (see reference):
  edge-softmax over incoming edges per dst node, attention-weighted
  gather of projected node features, segment-sum per dst, ELU, GRUCell.

Strategy (8 NeuronCores, SPMD, no collectives) — v3:
  * Host sorts edges by dst window (128 consecutive node ids); 392 windows
    are bin-packed (sorted by A-edge count, dealt 8-per-position) onto
    49 positions x 8 cores so each position's slot count is the max over
    only 8 windows instead of all 392 (~10% fewer padded slots).
  * Softmax shift-invariance: a_e = exp(l_e)/sum exp(l_e); the division by
    the segment denominator is folded through the segment sum:
    c_v = W @ (sum_e ex_e nf[src_e]) / (sum_e ex_e) + b.
  * The gather is latency-bound (~10 ns/row on one SWDGE queue).  It is
    split across all 4 SWDGE queues (ucode max) which hardware-parallelizes
    the descriptor streams (~5x), with gbufs=3 gather-tile rings so the
    descriptor generation for batch b+2 never stalls on batch b's
    consumers (keeps the queues' duty cycle high).
  * dma_gather needs int16 indices but V=50000 > 32767, so the nf table is
    addressed through two overlapping row views: A = rows [0, 32768)
    (src < 32768) and B = rows [17232, 50000) (idx = src - 17232).
    A rides queues {0,1}, B rides {2,3}.  Pad slots gather row 0 and are
    killed by dst_local = -1 in the one-hot.
  * Everything on-chip is bf16 (table, one-hot, weights, nf^T): DVE runs
    at 2x, PE matmuls get FWL weight loads, SBUF footprint halves.
    PSUM accumulation stays fp32 (rel err ~9e-3 < 2e-2).
  * Two-pass structure per iteration: pass 1 does gathers + one-hot builds
    + Gs=G*ex + per-window PE accumulation psum_ud += O.T @ [Gs|ex], then
    ctx = u/den straight into an SBUF ctx_all tile (so the PE never stalls
    mid-edge-stream on node-phase work).  Pass 2 (groups of GW=2 windows):
    PE transpose of ctx, cT = W_proj @ ctx^T, feature-major GRU so all
    biases are per-partition (folded into Act bias operands; zero bias
    matmuls), ELU's exp(x)-1 rewritten as 2t/(1-t) with t=tanh(x/2) so the
    whole node phase stays in the 'sigmoid_and_others' activation table
    (zero 1.3us table reloads), blend, relu, DMA out feature-major.
  * Output is [128, nodes] feature-major per core; host transposes and
    applies the inverse window permutation.
"""

import numpy as np

V, E, F = 50000, 800000, 128
NC = 8
WPC = 49              # windows per core
NPC = WPC * 128       # 6272 node slots per core
WTOT = NC * WPC       # 392 windows total
WPB = 2               # windows per gather batch
S_SPLIT = 32768       # src < S -> table A
OFF_B = V - 32768     # 17232; table B rows [OFF_B, V)

_compiled = {}


def _build_nc(T_win, sA=None, sB=None, skip_gather=False, skip_onehot=False,
              skip_mm=False, skip_node=False, repeat=1, one_act=False,
              n_q=1, sp=False, bf16_tab=False, n_calls=None):
    import concourse.bass as bass
    import concourse.bacc as bacc
    import concourse.mybir as mybir
    import concourse.tile as tile

    f32 = mybir.dt.float32
    bf16 = mybir.dt.bfloat16
    gdt = bf16 if bf16_tab else f32
    i16 = mybir.dt.int16
    AF = mybir.ActivationFunctionType
    OP = mybir.AluOpType
    AF_E = AF.Sigmoid if one_act else AF.Exp
    AF_T = AF.Sigmoid if one_act else AF.Tanh
    AF_R = AF.Sigmoid if one_act else AF.Relu

    if sA is None:
        sA, sB = T_win, 0   # legacy path unused
    SW = sA + sB            # slots per window
    T = WPC * SW            # tile-columns per core
    LA = WPC * sA * 128     # A-gather idx count per core
    LB = WPC * sB * 128

    nc = bacc.Bacc("TRN2", target_bir_lowering=False, debug=False,
                   num_devices=NC, num_swdge_queues=max(1, n_q))

    # ---- DRAM parameters ----
    idxa_d = nc.dram_tensor("idxa", [128, LA // 16], i16,
                            kind="ExternalInput")
    idxb_d = nc.dram_tensor("idxb", [128, LB // 16], i16,
                            kind="ExternalInput")
    dstla_d = nc.dram_tensor("dstla", [128, WPC * sA], f32,
                             kind="ExternalInput")
    dstlb_d = nc.dram_tensor("dstlb", [128, WPC * sB], f32,
                             kind="ExternalInput")
    logita_d = nc.dram_tensor("logita", [128, WPC * sA], f32,
                              kind="ExternalInput")
    logitb_d = nc.dram_tensor("logitb", [128, WPC * sB], f32,
                              kind="ExternalInput")
    table_d = nc.dram_tensor("table", [V, F], gdt, kind="ExternalInput")
    nfT_d = nc.dram_tensor("nfT", [128, NPC], f32, kind="ExternalInput")
    wprojT_d = nc.dram_tensor("wprojT", [128, 128], f32, kind="ExternalInput")
    wihT_d = nc.dram_tensor("wihT", [128, 384], f32, kind="ExternalInput")
    whhT_d = nc.dram_tensor("whhT", [128, 384], f32, kind="ExternalInput")
    bproj_d = nc.dram_tensor("bproj", [1, 128], f32, kind="ExternalInput")
    brz_d = nc.dram_tensor("brz", [1, 256], f32, kind="ExternalInput")
    bni_d = nc.dram_tensor("bni", [1, 128], f32, kind="ExternalInput")
    bnh_d = nc.dram_tensor("bnh", [1, 128], f32, kind="ExternalInput")
    iota_d = nc.dram_tensor("iota", [128, 128], f32, kind="ExternalInput")
    ident_d = nc.dram_tensor("ident", [128, 128], f32, kind="ExternalInput")
    onesc_d = nc.dram_tensor("onesc", [128, 1], f32, kind="ExternalInput")
    onesr_d = nc.dram_tensor("onesr", [1, 128], f32, kind="ExternalInput")
    tableb_d = nc.dram_tensor("tableb", [32768, 128], gdt,
                              kind="ExternalInput")
    out_d = nc.dram_tensor("out", [NPC, 128], f32, kind="ExternalOutput")

    tabA = table_d[0:32768, :]
    tabB = tableb_d[:]

    with tile.TileContext(nc) as tc:
        with (
            tc.tile_pool(name="const", bufs=1) as cpool,
            tc.tile_pool(name="gat", bufs=2) as gpool,
            tc.tile_pool(name="oh", bufs=2) as opool,
            tc.tile_pool(name="wrk", bufs=2) as wpool,
            tc.tile_pool(name="pedge", bufs=1, space="PSUM") as pe_pool,
            tc.tile_pool(name="pnode", bufs=1, space="PSUM") as pn_pool,
        ):
            def load(pool, name, dram, shape, dtype=f32):
                t = pool.tile(shape, dtype, tag=name)
                nc.sync.dma_start(t[:], dram[:])
                return t

            iota_sb = load(cpool, "iota", iota_d, [128, 128])
            ident_sb = load(cpool, "ident", ident_d, [128, 128])
            onesc_sb = load(cpool, "onesc", onesc_d, [128, 1])
            onesr_sb = load(cpool, "onesr", onesr_d, [1, 128])
            wproj_sb = load(cpool, "wproj", wprojT_d, [128, 128])
            wih_sb = load(cpool, "wih", wihT_d, [128, 384])
            whh_sb = load(cpool, "whh", whhT_d, [128, 384])
            bproj_sb = load(cpool, "bproj", bproj_d, [1, 128])
            brz_sb = load(cpool, "brz", brz_d, [1, 256])
            bni_sb = load(cpool, "bni", bni_d, [1, 128])
            bnh_sb = load(cpool, "bnh", bnh_d, [1, 128])
            idxa_sb = load(cpool, "idxa", idxa_d, [128, LA // 16], i16)
            idxb_sb = load(cpool, "idxb", idxb_d, [128, LB // 16], i16)
            dstla_sb = load(cpool, "dstla", dstla_d, [128, WPC * sA])
            dstlb_sb = load(cpool, "dstlb", dstlb_d, [128, WPC * sB])
            nfT_sb = load(cpool, "nfT", nfT_d, [128, NPC])

            exa_sb = cpool.tile([128, WPC * sA], f32, tag="exa")
            nc.sync.dma_start(exa_sb[:], logita_d[:])
            nc.scalar.activation(exa_sb[:], exa_sb[:], AF.Exp)
            exb_sb = cpool.tile([128, WPC * sB], f32, tag="exb")
            nc.sync.dma_start(exb_sb[:], logitb_d[:])
            nc.scalar.activation(exb_sb[:], exb_sb[:], AF.Exp)

            def apx(base, dims):
                return bass.AP(base.tensor, base.offset,
                               [list(base.ap[0])] + dims)

            n_batches = (WPC + WPB - 1) // WPB
            GA_static = GB_static = None
            if skip_gather:
                GA_static = cpool.tile([128, WPB * sA, 128], gdt, tag="GAs")
                nc.gpsimd.memset(GA_static[:], 0.0)
                GB_static = cpool.tile([128, WPB * sB, 128], gdt, tag="GBs")
                nc.gpsimd.memset(GB_static[:], 0.0)


            for _rep in range(repeat):
              for b in range(n_batches):
                w0 = b * WPB
                nw = min(WPB, WPC - w0)
                if skip_gather:
                    GA, GB = GA_static, GB_static
                else:
                    GA = gpool.tile([128, WPB * sA, 128], gdt, tag="GA")
                    GB = gpool.tile([128, WPB * sB, 128], gdt, tag="GB")

                    ncall = n_calls if n_calls else n_q

                    def qgather(G, tab, idx_sb, t0, nt):
                        # split [t0, t0+nt) tiles across ncall calls on n_q qs
                        per = (nt + ncall - 1) // ncall
                        q = 0
                        o = 0
                        while o < nt:
                            cn = min(per, nt - o)
                            ni = cn * 128
                            nc.gpsimd.dma_gather(
                                out_ap=G[:, o:o + cn, :],
                                in_ap=tab,
                                idxs_ap=idx_sb[:, ((t0 + o) * 128) // 16:
                                               ((t0 + o + cn) * 128) // 16],
                                num_idxs=ni, num_idxs_reg=ni, elem_size=128,
                                single_packet=sp, queue_num=q,
                            )
                            q = (q + 1) % max(1, n_q)
                            o += cn

                    qgather(GA, tabA, idxa_sb, w0 * sA, nw * sA)
                    qgather(GB, tabB, idxb_sb, w0 * sB, nw * sB)
                ntA, ntB = nw * sA, nw * sB
                cA0, cB0 = w0 * sA, w0 * sB
                OA = opool.tile([128, WPB * sA, 128], f32, tag="OA")
                OB = opool.tile([128, WPB * sB, 128], f32, tag="OB")
                GsA = gpool.tile([128, WPB * sA, 132], f32, tag="GsA")
                GsB = gpool.tile([128, WPB * sB, 132], f32, tag="GsB")
                if not skip_onehot:
                    for (O, dstl_sb, nt, c0) in (
                            (OA, dstla_sb, ntA, cA0),
                            (OB, dstlb_sb, ntB, cB0)):
                        nc.vector.tensor_tensor(
                            out=O[:, 0:nt, :],
                            in0=apx(iota_sb[:], [[0, nt], [1, 128]]),
                            in1=apx(dstl_sb[:, c0:c0 + nt],
                                    [[1, nt], [0, 128]]),
                            op=OP.is_equal)
                for (G, Gs, ex_sb, nt, c0) in (
                        (GA, GsA, exa_sb, ntA, cA0),
                        (GB, GsB, exb_sb, ntB, cB0)):
                    nc.vector.tensor_tensor(
                        out=Gs[:, 0:nt, 0:128], in0=G[:, 0:nt, :],
                        in1=apx(ex_sb[:, c0:c0 + nt], [[1, nt], [0, 128]]),
                        op=OP.mult)
                    nc.vector.tensor_copy(out=Gs[:, 0:nt, 128:129],
                                          in_=ex_sb[:, c0:c0 + nt])
                for wl in range(nw):
                    w = w0 + wl
                    psum_ud = pe_pool.tile([128, 132], f32, tag="psum_ud",
                                           bufs=2)
                    if not skip_mm:
                        for s_ in range(SW):
                            if s_ < sA:
                                Olh = OA[:, wl * sA + s_, :]
                                Grh = GsA[:, wl * sA + s_, 0:129]
                            else:
                                Olh = OB[:, wl * sB + (s_ - sA), :]
                                Grh = GsB[:, wl * sB + (s_ - sA), 0:129]
                            nc.tensor.matmul(
                                psum_ud[:, 0:129], lhsT=Olh, rhs=Grh,
                                start=(s_ == 0), stop=(s_ == SW - 1),
                            )

                    if skip_node:
                        continue
                    # ---- node phase for window w ----
                    den = wpool.tile([128, 1], f32, tag="den")
                    nc.vector.tensor_scalar(
                        out=den[:], in0=psum_ud[:, 128:129], scalar1=1e-30,
                        scalar2=None, op0=OP.max)
                    rec = wpool.tile([128, 1], f32, tag="rec")
                    nc.vector.reciprocal(rec[:], den[:])
                    ctx_t = wpool.tile([128, 128], f32, tag="ctx_t")
                    nc.vector.tensor_scalar(
                        out=ctx_t[:], in0=psum_ud[:, 0:128],
                        scalar1=rec[:, 0:1],
                        scalar2=None, op0=OP.mult)

                    ptr = pn_pool.tile([128, 128], f32, tag="ptr", bufs=2)
                    nc.tensor.transpose(ptr[:], ctx_t[:], ident_sb[:])
                    ctxT = wpool.tile([128, 128], f32, tag="ctxT")
                    nc.vector.tensor_copy(out=ctxT[:], in_=ptr[:])

                    # cT = W_proj @ ctx~.T + b_proj  (H on partitions)
                    psum_cT = pn_pool.tile([128, 128], f32, tag="psum_cT",
                                           bufs=2)
                    nc.tensor.matmul(psum_cT[:], lhsT=wproj_sb[:],
                                     rhs=ctxT[:], start=True, stop=False)
                    nc.tensor.matmul(psum_cT[:], lhsT=bproj_sb[:],
                                     rhs=onesr_sb[:], start=False, stop=True)

                    # elu(cT) = max(cT,0) + exp(min(cT,0)) - 1
                    cmin = wpool.tile([128, 128], f32, tag="cmin")
                    nc.vector.tensor_scalar(out=cmin[:], in0=psum_cT[:],
                                            scalar1=0.0, scalar2=None,
                                            op0=OP.min)
                    cexp = wpool.tile([128, 128], f32, tag="cexp")
                    nc.scalar.activation(cexp[:], cmin[:], AF_E)
                    crelu = wpool.tile([128, 128], f32, tag="crelu")
                    nc.vector.tensor_scalar(out=crelu[:], in0=psum_cT[:],
                                            scalar1=0.0, scalar2=None,
                                            op0=OP.max)
                    ce1 = wpool.tile([128, 128], f32, tag="ce1")
                    nc.vector.tensor_scalar(out=ce1[:], in0=cexp[:],
                                            scalar1=1.0, scalar2=None,
                                            op0=OP.subtract)
                    ctxT2 = wpool.tile([128, 128], f32, tag="ctxT2")
                    nc.vector.tensor_tensor(out=ctxT2[:], in0=ce1[:],
                                            in1=crelu[:], op=OP.add)

                    nfT_tile = nfT_sb[:, w * 128:(w + 1) * 128]
                    # gates PSUM: [0:256]=r|z (gi+gh), [256:384]=i_n,
                    # [384:512]=h_n
                    psum_g = pn_pool.tile([128, 512], f32, tag="psum_g",
                                          bufs=2)
                    psum_rz = psum_g[:, 0:256]
                    nc.tensor.matmul(psum_rz, lhsT=ctxT2[:],
                                     rhs=wih_sb[:, 0:256],
                                     start=True, stop=False)
                    nc.tensor.matmul(psum_rz, lhsT=nfT_tile,
                                     rhs=whh_sb[:, 0:256],
                                     start=False, stop=False)
                    nc.tensor.matmul(psum_rz, lhsT=onesr_sb[:],
                                     rhs=brz_sb[:], start=False, stop=True)
                    psum_nh = psum_g[:, 256:512]
                    nc.tensor.matmul(psum_nh[:, 0:128], lhsT=ctxT2[:],
                                     rhs=wih_sb[:, 256:384],
                                     start=True, stop=False)
                    nc.tensor.matmul(psum_nh[:, 0:128], lhsT=onesr_sb[:],
                                     rhs=bni_sb[:], start=False, stop=True)
                    nc.tensor.matmul(psum_nh[:, 128:256], lhsT=nfT_tile,
                                     rhs=whh_sb[:, 256:384],
                                     start=True, stop=False)
                    nc.tensor.matmul(psum_nh[:, 128:256], lhsT=onesr_sb[:],
                                     rhs=bnh_sb[:], start=False, stop=True)

                    rzs = wpool.tile([128, 256], f32, tag="rzs")
                    nc.scalar.activation(rzs[:], psum_rz, AF.Sigmoid)
                    nt1 = wpool.tile([128, 128], f32, tag="nt1")
                    nc.vector.tensor_tensor(out=nt1[:], in0=rzs[:, 0:128],
                                            in1=psum_nh[:, 128:256],
                                            op=OP.mult)
                    nt2 = wpool.tile([128, 128], f32, tag="nt2")
                    nc.vector.tensor_tensor(out=nt2[:], in0=nt1[:],
                                            in1=psum_nh[:, 0:128],
                                            op=OP.add)
                    nn = wpool.tile([128, 128], f32, tag="nn")
                    nc.scalar.activation(nn[:], nt2[:], AF_T)

                    pnf = pn_pool.tile([128, 128], f32, tag="ptr", bufs=2)
                    nc.tensor.transpose(pnf[:], nfT_tile, ident_sb[:])
                    df = wpool.tile([128, 128], f32, tag="df")
                    nc.vector.tensor_tensor(out=df[:], in0=pnf[:], in1=nn[:],
                                            op=OP.subtract)
                    dz = wpool.tile([128, 128], f32, tag="dz")
                    nc.vector.tensor_tensor(out=dz[:], in0=df[:],
                                            in1=rzs[:, 128:256], op=OP.mult)
                    hh = wpool.tile([128, 128], f32, tag="hh")
                    nc.vector.tensor_tensor(out=hh[:], in0=dz[:], in1=nn[:],
                                            op=OP.add)
                    outt = wpool.tile([128, 128], f32, tag="outt")
                    nc.scalar.activation(outt[:], hh[:], AF_R)
                    nc.sync.dma_start(out_d[w * 128:(w + 1) * 128, :],
                                      outt[:])

    nc.compile()
    return nc


def _build_v2(sA, sB, repeat=1, n_q=4, WPBv=4, GW=2, skip_gather=False,
              skip_onehot=False, skip_mm=False, skip_node=False):
    """bf16 edge phase + feature-major node phase, 4-queue gathers."""
    import concourse.bass as bass
    import concourse.bacc as bacc
    import concourse.mybir as mybir
    import concourse.tile as tile

    f32 = mybir.dt.float32
    bf16 = mybir.dt.bfloat16
    i16 = mybir.dt.int16
    AF = mybir.ActivationFunctionType
    OP = mybir.AluOpType

    SW = sA + sB
    LA = WPC * sA * 128
    LB = WPC * sB * 128

    nc = bacc.Bacc("TRN2", target_bir_lowering=False, debug=False,
                   num_devices=NC, num_swdge_queues=max(1, n_q))

    idxa_d = nc.dram_tensor("idxa", [128, LA // 16], i16,
                            kind="ExternalInput")
    idxb_d = nc.dram_tensor("idxb", [128, LB // 16], i16,
                            kind="ExternalInput")
    dstla_d = nc.dram_tensor("dstla", [128, WPC * sA], bf16,
                             kind="ExternalInput")
    dstlb_d = nc.dram_tensor("dstlb", [128, WPC * sB], bf16,
                             kind="ExternalInput")
    logita_d = nc.dram_tensor("logita", [128, WPC * sA], f32,
                              kind="ExternalInput")
    logitb_d = nc.dram_tensor("logitb", [128, WPC * sB], f32,
                              kind="ExternalInput")
    table_d = nc.dram_tensor("table", [V, F], bf16, kind="ExternalInput")
    tableb_d = nc.dram_tensor("tableb", [32768, 128], bf16,
                              kind="ExternalInput")
    nfT_d = nc.dram_tensor("nfT", [128, NPC], bf16, kind="ExternalInput")
    wprojT_d = nc.dram_tensor("wprojT", [128, 128], bf16,
                              kind="ExternalInput")
    wihT_d = nc.dram_tensor("wihT", [128, 384], bf16, kind="ExternalInput")
    whhT_d = nc.dram_tensor("whhT", [128, 384], bf16, kind="ExternalInput")
    ident_d = nc.dram_tensor("ident", [128, 128], bf16, kind="ExternalInput")
    iota_d = nc.dram_tensor("iota", [128, 128], bf16, kind="ExternalInput")
    bcols_d = nc.dram_tensor("bcols", [128, 5], f32, kind="ExternalInput")
    out_d = nc.dram_tensor("out", [128, NPC], f32, kind="ExternalOutput")

    tabA = table_d[0:32768, :]
    tabB = tableb_d[:]

    with tile.TileContext(nc) as tc:
        with (
            tc.tile_pool(name="const", bufs=1) as cpool,
            tc.tile_pool(name="gat", bufs=2) as gpool,
            tc.tile_pool(name="oh", bufs=2) as opool,
            tc.tile_pool(name="wrk", bufs=2) as wpool,
            tc.tile_pool(name="pedge", bufs=1, space="PSUM") as pe_pool,
            tc.tile_pool(name="pnode", bufs=1, space="PSUM") as pn_pool,
        ):
            def load(pool, name, dram, shape, dtype=f32):
                t = pool.tile(shape, dtype, tag=name)
                nc.sync.dma_start(t[:], dram[:])
                return t

            iota_sb = load(cpool, "iota", iota_d, [128, 128], bf16)
            ident_sb = load(cpool, "ident", ident_d, [128, 128], bf16)
            wproj_sb = load(cpool, "wproj", wprojT_d, [128, 128], bf16)
            wih_sb = load(cpool, "wih", wihT_d, [128, 384], bf16)
            whh_sb = load(cpool, "whh", whhT_d, [128, 384], bf16)
            bcols_sb = load(cpool, "bcols", bcols_d, [128, 5], f32)
            bproj_c = bcols_sb[:, 0:1]
            br_c = bcols_sb[:, 1:2]
            bz_c = bcols_sb[:, 2:3]
            bni_c = bcols_sb[:, 3:4]
            bnh_c = bcols_sb[:, 4:5]
            idxa_sb = load(cpool, "idxa", idxa_d, [128, LA // 16], i16)
            idxb_sb = load(cpool, "idxb", idxb_d, [128, LB // 16], i16)
            dstla_sb = load(cpool, "dstla", dstla_d, [128, WPC * sA], bf16)
            dstlb_sb = load(cpool, "dstlb", dstlb_d, [128, WPC * sB], bf16)
            nfT_sb = load(cpool, "nfT", nfT_d, [128, NPC], bf16)

            lstage = cpool.tile([128, WPC * sA], f32, tag="lstage")
            exa_sb = cpool.tile([128, WPC * sA], bf16, tag="exa")
            nc.sync.dma_start(lstage[:], logita_d[:])
            nc.scalar.activation(exa_sb[:], lstage[:], AF.Exp)
            lstageb = cpool.tile([128, WPC * sB], f32, tag="lstageb")
            exb_sb = cpool.tile([128, WPC * sB], bf16, tag="exb")
            nc.sync.dma_start(lstageb[:], logitb_d[:])
            nc.scalar.activation(exb_sb[:], lstageb[:], AF.Exp)

            def apx(base, dims):
                return bass.AP(base.tensor, base.offset,
                               [list(base.ap[0])] + dims)

            n_batches = (WPC + WPBv - 1) // WPBv
            GA_static = GB_static = None
            if skip_gather:
                GA_static = cpool.tile([128, WPBv * sA, 128], bf16,
                                       tag="GAs")
                nc.gpsimd.memset(GA_static[:], 0.0)
                GB_static = cpool.tile([128, WPBv * sB, 128], bf16,
                                       tag="GBs")
                nc.gpsimd.memset(GB_static[:], 0.0)

            for _rep in range(repeat):
              for b in range(n_batches):
                w0 = b * WPBv
                nw = min(WPBv, WPC - w0)
                if skip_gather:
                    GA, GB = GA_static, GB_static
                else:
                    GA = gpool.tile([128, WPBv * sA, 128], bf16, tag="GA")
                    GB = gpool.tile([128, WPBv * sB, 128], bf16, tag="GB")

                    def qgather(G, tab, idx_sb, t0, nt, q0):
                        half = (nt + 1) // 2
                        for i, (o, cn) in enumerate(
                                ((0, half), (half, nt - half))):
                            if cn <= 0:
                                continue
                            ni = cn * 128
                            nc.gpsimd.dma_gather(
                                out_ap=G[:, o:o + cn, :],
                                in_ap=tab,
                                idxs_ap=idx_sb[:, ((t0 + o) * 128) // 16:
                                               ((t0 + o + cn) * 128) // 16],
                                num_idxs=ni, num_idxs_reg=ni, elem_size=128,
                                single_packet=False,
                                queue_num=(q0 + i) % max(1, n_q),
                            )

                    qgather(GA, tabA, idxa_sb, w0 * sA, nw * sA, 0)
                    qgather(GB, tabB, idxb_sb, w0 * sB, nw * sB,
                            2 % max(1, n_q))
                ntA, ntB = nw * sA, nw * sB
                cA0, cB0 = w0 * sA, w0 * sB
                OA = opool.tile([128, WPBv * sA, 128], bf16, tag="OA")
                OB = opool.tile([128, WPBv * sB, 128], bf16, tag="OB")
                GsA = gpool.tile([128, WPBv * sA, 132], bf16, tag="GsA")
                GsB = gpool.tile([128, WPBv * sB, 132], bf16, tag="GsB")
                if not skip_onehot:
                    for (O, dstl_sb, nt, c0) in (
                            (OA, dstla_sb, ntA, cA0),
                            (OB, dstlb_sb, ntB, cB0)):
                        nc.vector.tensor_tensor(
                            out=O[:, 0:nt, :],
                            in0=apx(iota_sb[:], [[0, nt], [1, 128]]),
                            in1=apx(dstl_sb[:, c0:c0 + nt],
                                    [[1, nt], [0, 128]]),
                            op=OP.is_equal)
                for (G, Gs, ex_sb, nt, c0) in (
                        (GA, GsA, exa_sb, ntA, cA0),
                        (GB, GsB, exb_sb, ntB, cB0)):
                    nc.vector.tensor_tensor(
                        out=Gs[:, 0:nt, 0:128], in0=G[:, 0:nt, :],
                        in1=apx(ex_sb[:, c0:c0 + nt], [[1, nt], [0, 128]]),
                        op=OP.mult)
                    nc.vector.tensor_copy(out=Gs[:, 0:nt, 128:129],
                                          in_=ex_sb[:, c0:c0 + nt])

                for g0 in range(0, nw, GW):
                    ng = min(GW, nw - g0)
                    gn = ng * 128
                    ctxTg = wpool.tile([128, GW * 128], bf16, tag="ctxTg")
                    for wl in range(g0, g0 + ng):
                        w = w0 + wl
                        psum_ud = pe_pool.tile([128, 132], f32,
                                               tag="psum_ud", bufs=2)
                        if not skip_mm:
                            for s_ in range(SW):
                                if s_ < sA:
                                    Olh = OA[:, wl * sA + s_, :]
                                    Grh = GsA[:, wl * sA + s_, 0:129]
                                else:
                                    Olh = OB[:, wl * sB + (s_ - sA), :]
                                    Grh = GsB[:, wl * sB + (s_ - sA), 0:129]
                                nc.tensor.matmul(
                                    psum_ud[:, 0:129], lhsT=Olh, rhs=Grh,
                                    start=(s_ == 0), stop=(s_ == SW - 1),
                                )
                        if skip_node:
                            continue
                        # ---- per-window: ctx = u/den, transpose ----
                        den = wpool.tile([128, 1], f32, tag="den")
                        nc.vector.tensor_scalar(
                            out=den[:], in0=psum_ud[:, 128:129],
                            scalar1=1e-30, scalar2=None, op0=OP.max)
                        rec = wpool.tile([128, 1], f32, tag="rec")
                        nc.vector.reciprocal(rec[:], den[:])
                        ctx_t = wpool.tile([128, 128], bf16, tag="ctx_t")
                        nc.vector.tensor_scalar(
                            out=ctx_t[:], in0=psum_ud[:, 0:128],
                            scalar1=rec[:, 0:1], scalar2=None, op0=OP.mult)
                        ptr = pn_pool.tile([128, 128], bf16, tag="ptr",
                                           bufs=1)
                        nc.tensor.transpose(ptr[:], ctx_t[:], ident_sb[:])
                        nc.vector.tensor_copy(
                            out=ctxTg[:, (wl - g0) * 128:(wl - g0 + 1) * 128],
                            in_=ptr[:])

                    if skip_node:
                        continue
                    # ---- group node phase (f-major) ----
                    psum_cT = pn_pool.tile([128, GW * 128], f32,
                                           tag="psum_cT", bufs=1)
                    nc.tensor.matmul(psum_cT[:, 0:gn], lhsT=wproj_sb[:],
                                     rhs=ctxTg[:, 0:gn], start=True,
                                     stop=True)
                    # ELU(x+bproj) via tanh: e^x-1 = 2t/(1-t), t=tanh(x/2)
                    cmin = wpool.tile([128, GW * 128], f32, tag="cmin")
                    nc.vector.tensor_scalar(
                        out=cmin[:, 0:gn], in0=psum_cT[:, 0:gn],
                        scalar1=bproj_c, scalar2=0.0, op0=OP.add,
                        op1=OP.min)
                    th = wpool.tile([128, GW * 128], f32, tag="th")
                    nc.scalar.activation(th[:, 0:gn], cmin[:, 0:gn],
                                         AF.Tanh, scale=0.5)
                    omt = wpool.tile([128, GW * 128], f32, tag="omt")
                    nc.vector.tensor_scalar(
                        out=omt[:, 0:gn], in0=th[:, 0:gn], scalar1=-1.0,
                        scalar2=1.0, op0=OP.mult, op1=OP.add)
                    rv = wpool.tile([128, GW * 128], f32, tag="rv")
                    nc.vector.reciprocal(rv[:, 0:gn], omt[:, 0:gn])
                    eneg = wpool.tile([128, GW * 128], f32, tag="eneg")
                    nc.vector.scalar_tensor_tensor(
                        out=eneg[:, 0:gn], in0=th[:, 0:gn], scalar=2.0,
                        in1=rv[:, 0:gn], op0=OP.mult, op1=OP.mult)
                    crelu = wpool.tile([128, GW * 128], f32, tag="crelu")
                    nc.vector.tensor_scalar(
                        out=crelu[:, 0:gn], in0=psum_cT[:, 0:gn],
                        scalar1=bproj_c, scalar2=0.0, op0=OP.add,
                        op1=OP.max)
                    ctx2 = wpool.tile([128, GW * 128], bf16, tag="ctx2")
                    nc.vector.tensor_tensor(
                        out=ctx2[:, 0:gn], in0=eneg[:, 0:gn],
                        in1=crelu[:, 0:gn], op=OP.add)

                    nfTg = nfT_sb[:, (w0 + g0) * 128:(w0 + g0 + ng) * 128]
                    psum_g = pn_pool.tile([128, GW * 512], f32,
                                          tag="psum_g", bufs=2)
                    psum_rz = psum_g[:, 0:GW * 256]
                    psum_nh = psum_g[:, GW * 256:GW * 512]
                    GWn = GW * 128
                    nc.tensor.matmul(psum_rz[:, 0:gn],
                                     lhsT=wih_sb[:, 0:128],
                                     rhs=ctx2[:, 0:gn],
                                     start=True, stop=False)
                    nc.tensor.matmul(psum_rz[:, 0:gn],
                                     lhsT=whh_sb[:, 0:128], rhs=nfTg,
                                     start=False, stop=True)
                    nc.tensor.matmul(psum_rz[:, GWn:GWn + gn],
                                     lhsT=wih_sb[:, 128:256],
                                     rhs=ctx2[:, 0:gn],
                                     start=True, stop=False)
                    nc.tensor.matmul(psum_rz[:, GWn:GWn + gn],
                                     lhsT=whh_sb[:, 128:256], rhs=nfTg,
                                     start=False, stop=True)
                    nc.tensor.matmul(psum_nh[:, 0:gn],
                                     lhsT=wih_sb[:, 256:384],
                                     rhs=ctx2[:, 0:gn],
                                     start=True, stop=True)
                    nc.tensor.matmul(psum_nh[:, GWn:GWn + gn],
                                     lhsT=whh_sb[:, 256:384], rhs=nfTg,
                                     start=True, stop=True)

                    sig_r = wpool.tile([128, GW * 128], f32, tag="sig_r")
                    nc.scalar.activation(sig_r[:, 0:gn], psum_rz[:, 0:gn],
                                         AF.Sigmoid, bias=br_c)
                    sig_z = wpool.tile([128, GW * 128], bf16, tag="sig_z")
                    nc.scalar.activation(sig_z[:, 0:gn],
                                         psum_rz[:, GWn:GWn + gn],
                                         AF.Sigmoid, bias=bz_c)
                    hnr = wpool.tile([128, GW * 128], f32, tag="hnr")
                    nc.vector.scalar_tensor_tensor(
                        out=hnr[:, 0:gn], in0=psum_nh[:, GWn:GWn + gn],
                        scalar=bnh_c, in1=sig_r[:, 0:gn],
                        op0=OP.add, op1=OP.mult)
                    npre = wpool.tile([128, GW * 128], f32, tag="npre")
                    nc.vector.tensor_tensor(
                        out=npre[:, 0:gn], in0=hnr[:, 0:gn],
                        in1=psum_nh[:, 0:gn], op=OP.add)
                    nn = wpool.tile([128, GW * 128], bf16, tag="nn")
                    nc.scalar.activation(nn[:, 0:gn], npre[:, 0:gn],
                                         AF.Tanh, bias=bni_c)
                    df = wpool.tile([128, GW * 128], bf16, tag="df")
                    nc.vector.tensor_tensor(
                        out=df[:, 0:gn], in0=nfTg, in1=nn[:, 0:gn],
                        op=OP.subtract)
                    dz = wpool.tile([128, GW * 128], bf16, tag="dz")
                    nc.vector.tensor_tensor(
                        out=dz[:, 0:gn], in0=df[:, 0:gn],
                        in1=sig_z[:, 0:gn], op=OP.mult)
                    hh = wpool.tile([128, GW * 128], bf16, tag="hh")
                    nc.vector.tensor_tensor(
                        out=hh[:, 0:gn], in0=dz[:, 0:gn],
                        in1=nn[:, 0:gn], op=OP.add)
                    outg = wpool.tile([128, GW * 128], f32, tag="outg")
                    nc.vector.tensor_scalar(
                        out=outg[:, 0:gn], in0=hh[:, 0:gn], scalar1=0.0,
                        scalar2=None, op0=OP.max)
                    nc.sync.dma_start(
                        out_d[:, (w0 + g0) * 128:(w0 + g0 + ng) * 128],
                        outg[:, 0:gn])

    nc.compile()
    return nc


def _prep(edge_logits, node_feats, W_proj, b_proj, w_ih, w_hh, b_ih, b_hh,
          src, dst, bf16_tab=False):
    """Host-side sharding. Returns (T_win, sA, sB, in_maps)."""
    logits = np.asarray(edge_logits, np.float32).reshape(-1)
    src = np.asarray(src, np.int64)
    dst = np.asarray(dst, np.int64)

    is_b = (src >= S_SPLIT).astype(np.int64)
    win = dst // 128
    key = win * 2 + is_b
    order = np.argsort(key, kind="stable")
    key_s = key[order]
    src_s = src[order]
    dst_s = dst[order]
    log_s = logits[order]

    counts = np.bincount(key_s, minlength=WTOT * 2)
    cA = counts[0::2]
    cB = counts[1::2]
    sA = int((cA.max() + 127) // 128)
    sB = int((cB.max() + 127) // 128)
    T_win = sA + sB

    starts = np.zeros(WTOT * 2, np.int64)
    starts[1:] = np.cumsum(counts)[:-1]
    pos = np.arange(E, dtype=np.int64) - starts[key_s]

    # flat slot index within the core-ordered [WTOT, sA*128 | sB*128] arrays
    winv = key_s // 2
    grp = key_s % 2
    idxA = np.zeros(WTOT * sA * 128, np.int16)
    idxB = np.zeros(WTOT * sB * 128, np.int16)
    dstlA = np.full(WTOT * sA * 128, -1.0, np.float32)
    dstlB = np.full(WTOT * sB * 128, -1.0, np.float32)
    logA = np.zeros(WTOT * sA * 128, np.float32)
    logB = np.zeros(WTOT * sB * 128, np.float32)

    mA = grp == 0
    mB = ~mA
    flatA = winv[mA] * (sA * 128) + pos[mA]
    flatB = winv[mB] * (sB * 128) + pos[mB]
    idxA[flatA] = src_s[mA].astype(np.int16)
    idxB[flatB] = (src_s[mB] - OFF_B).astype(np.int16)
    dstlA[flatA] = (dst_s[mA] - winv[mA] * 128).astype(np.float32)
    dstlB[flatB] = (dst_s[mB] - winv[mB] * 128).astype(np.float32)
    logA[flatA] = log_s[mA]
    logB[flatB] = log_s[mB]

    def core_tiles(a, slots):
        a = a.reshape(WTOT, slots, 128)
        return [np.ascontiguousarray(
            a[k * WPC:(k + 1) * WPC].transpose(2, 0, 1)
            .reshape(128, WPC * slots)) for k in range(NC)]

    dstlA_cores = core_tiles(dstlA, sA)
    dstlB_cores = core_tiles(dstlB, sB)
    logA_cores = core_tiles(logA, sA)
    logB_cores = core_tiles(logB, sB)

    def core_idx(a, slots):
        a = a.reshape(WTOT, slots * 128)
        out = []
        for k in range(NC):
            flat = a[k * WPC:(k + 1) * WPC].reshape(-1)
            blk = flat.reshape(-1, 16).T      # [16, L/16], i -> [i%16,i//16]
            out.append(np.ascontiguousarray(np.tile(blk, (8, 1))))
        return out

    idxA_cores = core_idx(idxA, sA)
    idxB_cores = core_idx(idxB, sB)

    nf = np.asarray(node_feats, np.float32)
    nf_pad = np.zeros((NC * NPC, F), np.float32)
    nf_pad[:V] = nf

    if bf16_tab:
        import ml_dtypes
        table = np.ascontiguousarray(nf.astype(ml_dtypes.bfloat16))
        tableb = np.ascontiguousarray(table[OFF_B:])
    else:
        table = np.ascontiguousarray(nf)
        tableb = np.ascontiguousarray(nf[OFF_B:])
    wprojT = np.ascontiguousarray(np.asarray(W_proj, np.float32).T)
    wihT = np.ascontiguousarray(np.asarray(w_ih, np.float32).T)
    whhT = np.ascontiguousarray(np.asarray(w_hh, np.float32).T)
    bproj = np.asarray(b_proj, np.float32).reshape(1, 128)
    bih = np.asarray(b_ih, np.float32).reshape(384)
    bhh = np.asarray(b_hh, np.float32).reshape(384)
    brz = (bih[0:256] + bhh[0:256]).reshape(1, 256)
    bni = bih[256:384].reshape(1, 128)
    bnh = bhh[256:384].reshape(1, 128)
    iota = np.tile(np.arange(128, dtype=np.float32), (128, 1))
    ident = np.eye(128, dtype=np.float32)
    onesc = np.ones((128, 1), np.float32)
    onesr = np.ones((1, 128), np.float32)

    in_maps = []
    for k in range(NC):
        sl = nf_pad[k * NPC:(k + 1) * NPC]
        nfT = np.ascontiguousarray(sl.T)
        in_maps.append({
            "idxa": idxA_cores[k], "idxb": idxB_cores[k],
            "dstla": dstlA_cores[k], "dstlb": dstlB_cores[k],
            "logita": logA_cores[k], "logitb": logB_cores[k],
            "table": table, "tableb": tableb,
            "nfT": nfT,
            "wprojT": wprojT, "wihT": wihT, "whhT": whhT,
            "bproj": bproj, "brz": brz, "bni": bni, "bnh": bnh,
            "iota": iota, "ident": ident,
            "onesc": onesc, "onesr": onesr,
        })
    return T_win, sA, sB, in_maps


def _build_v3(sAl, sBl, repeat=1, n_q=4, WPBv=4, GW=2, skip_gather=False,
              skip_onehot=False, skip_mm=False, skip_node=False,
              probe=None, gbufs=2, qmode="ab", streami=False, wb=2,
              il=False):
    """Two-pass: edge phase stores ctx to SBUF; node pass runs after.

    sAl/sBl: per-position slot counts (len WPC) from host bin-packing.
    """
    import concourse.bass as bass
    import concourse.bacc as bacc
    import concourse.mybir as mybir
    import concourse.tile as tile

    f32 = mybir.dt.float32
    bf16 = mybir.dt.bfloat16
    i16 = mybir.dt.int16
    AF = mybir.ActivationFunctionType
    OP = mybir.AluOpType

    if probe == "gather":
        skip_onehot = skip_mm = skip_node = True
    elif probe == "edge":
        skip_mm = skip_node = True
    offA = [0]
    offB = [0]
    for j in range(WPC):
        offA.append(offA[-1] + sAl[j])
        offB.append(offB[-1] + sBl[j])
    ncA, ncB = offA[-1], offB[-1]
    LA, LB = ncA * 128, ncB * 128
    n_batches = (WPC + WPBv - 1) // WPBv
    maxbA = max(offA[min(b * WPBv + WPBv, WPC)] - offA[b * WPBv]
                for b in range(n_batches))
    maxbB = max(offB[min(b * WPBv + WPBv, WPC)] - offB[b * WPBv]
                for b in range(n_batches))

    nc = bacc.Bacc("TRN2", target_bir_lowering=False, debug=False,
                   num_devices=NC, num_swdge_queues=max(1, n_q))

    idxa_d = nc.dram_tensor("idxa", [128, LA // 16], i16,
                            kind="ExternalInput")
    idxb_d = nc.dram_tensor("idxb", [128, LB // 16], i16,
                            kind="ExternalInput")
    dstla_d = nc.dram_tensor("dstla", [128, ncA], bf16,
                             kind="ExternalInput")
    dstlb_d = nc.dram_tensor("dstlb", [128, ncB], bf16,
                             kind="ExternalInput")
    logita_d = nc.dram_tensor("logita", [128, ncA], f32,
                              kind="ExternalInput")
    logitb_d = nc.dram_tensor("logitb", [128, ncB], f32,
                              kind="ExternalInput")
    table_d = nc.dram_tensor("table", [V, F], bf16, kind="ExternalInput")
    tableb_d = nc.dram_tensor("tableb", [32768, 128], bf16,
                              kind="ExternalInput")
    nfT_d = nc.dram_tensor("nfT", [128, NPC], bf16, kind="ExternalInput")
    wprojT_d = nc.dram_tensor("wprojT", [128, 128], bf16,
                              kind="ExternalInput")
    wihT_d = nc.dram_tensor("wihT", [128, 384], bf16, kind="ExternalInput")
    whhT_d = nc.dram_tensor("whhT", [128, 384], bf16, kind="ExternalInput")
    ident_d = nc.dram_tensor("ident", [128, 128], bf16, kind="ExternalInput")
    iota_d = nc.dram_tensor("iota", [128, 128], bf16, kind="ExternalInput")
    bcols_d = nc.dram_tensor("bcols", [128, 5], f32, kind="ExternalInput")
    out_d = nc.dram_tensor("out", [128, NPC], f32, kind="ExternalOutput")

    tabA = table_d[0:32768, :]
    tabB = tableb_d[:]

    with tile.TileContext(nc) as tc:
        with (
            tc.tile_pool(name="const", bufs=1) as cpool,
            tc.tile_pool(name="ctxp", bufs=2) as xpool,
            tc.tile_pool(name="gat", bufs=2) as gpool,
            tc.tile_pool(name="oh", bufs=2) as opool,
            tc.tile_pool(name="wrk", bufs=2) as wpool,
            tc.tile_pool(name="pedge", bufs=1, space="PSUM") as pe_pool,
            tc.tile_pool(name="pnode", bufs=1, space="PSUM") as pn_pool,
        ):
            def load(pool, name, dram, shape, dtype=f32):
                t = pool.tile(shape, dtype, tag=name)
                nc.sync.dma_start(t[:], dram[:])
                return t

            iota_sb = load(cpool, "iota", iota_d, [128, 128], bf16)
            ident_sb = load(cpool, "ident", ident_d, [128, 128], bf16)
            wproj_sb = load(cpool, "wproj", wprojT_d, [128, 128], bf16)
            wih_sb = load(cpool, "wih", wihT_d, [128, 384], bf16)
            whh_sb = load(cpool, "whh", whhT_d, [128, 384], bf16)
            bcols_sb = load(cpool, "bcols", bcols_d, [128, 5], f32)
            bproj_c = bcols_sb[:, 0:1]
            br_c = bcols_sb[:, 1:2]
            bz_c = bcols_sb[:, 2:3]
            bni_c = bcols_sb[:, 3:4]
            bnh_c = bcols_sb[:, 4:5]
            if not streami:
                idxa_sb = load(cpool, "idxa", idxa_d, [128, LA // 16], i16)
                idxb_sb = load(cpool, "idxb", idxb_d, [128, LB // 16], i16)
            dstla_sb = load(cpool, "dstla", dstla_d, [128, ncA], bf16)
            dstlb_sb = load(cpool, "dstlb", dstlb_d, [128, ncB], bf16)
            nfT_sb = load(cpool, "nfT", nfT_d, [128, NPC], bf16)

            lstage = cpool.tile([128, ncA], f32, tag="lstage")
            exa_sb = cpool.tile([128, ncA], bf16, tag="exa")
            nc.sync.dma_start(lstage[:], logita_d[:])
            nc.scalar.activation(exa_sb[:], lstage[:], AF.Exp)
            lstageb = cpool.tile([128, ncB], f32, tag="lstageb")
            exb_sb = cpool.tile([128, ncB], bf16, tag="exb")
            nc.sync.dma_start(lstageb[:], logitb_d[:])
            nc.scalar.activation(exb_sb[:], lstageb[:], AF.Exp)

            def apx(base, dims):
                return bass.AP(base.tensor, base.offset,
                               [list(base.ap[0])] + dims)

            GA_static = GB_static = None
            if skip_gather:
                GA_static = cpool.tile([128, maxbA, 128], bf16, tag="GAs")
                nc.gpsimd.memset(GA_static[:], 0.0)
                GB_static = cpool.tile([128, maxbB, 128], bf16, tag="GBs")
                nc.gpsimd.memset(GB_static[:], 0.0)

            def node_group(g0, ctx_all):
                    ng = min(GW, WPC - g0)
                    gn = ng * 128
                    GWn = GW * 128
                    ctxTg = wpool.tile([128, GW * 128], bf16, tag="ctxTg")
                    for wl in range(ng):
                        ptr = pn_pool.tile([128, 128], bf16, tag="ptr",
                                           bufs=1)
                        nc.tensor.transpose(
                            ptr[:],
                            ctx_all[:, (g0 + wl) * 128:(g0 + wl + 1) * 128],
                            ident_sb[:])
                        nc.vector.tensor_copy(
                            out=ctxTg[:, wl * 128:(wl + 1) * 128],
                            in_=ptr[:])

                    psum_cT = pn_pool.tile([128, GW * 128], f32,
                                           tag="psum_cT", bufs=1)
                    nc.tensor.matmul(psum_cT[:, 0:gn], lhsT=wproj_sb[:],
                                     rhs=ctxTg[:, 0:gn], start=True,
                                     stop=True)
                    cmin = wpool.tile([128, GW * 128], f32, tag="cmin",
                                      bufs=wb)
                    nc.vector.tensor_scalar(
                        out=cmin[:, 0:gn], in0=psum_cT[:, 0:gn],
                        scalar1=bproj_c, scalar2=0.0, op0=OP.add,
                        op1=OP.min)
                    th = wpool.tile([128, GW * 128], f32, tag="th",
                                    bufs=wb)
                    nc.scalar.activation(th[:, 0:gn], cmin[:, 0:gn],
                                         AF.Tanh, scale=0.5)
                    omt = wpool.tile([128, GW * 128], f32, tag="omt",
                                     bufs=wb)
                    nc.vector.tensor_scalar(
                        out=omt[:, 0:gn], in0=th[:, 0:gn], scalar1=-1.0,
                        scalar2=1.0, op0=OP.mult, op1=OP.add)
                    rv = wpool.tile([128, GW * 128], f32, tag="rv",
                                    bufs=wb)
                    nc.vector.reciprocal(rv[:, 0:gn], omt[:, 0:gn])
                    eneg = wpool.tile([128, GW * 128], f32, tag="eneg",
                                      bufs=wb)
                    nc.vector.scalar_tensor_tensor(
                        out=eneg[:, 0:gn], in0=th[:, 0:gn], scalar=2.0,
                        in1=rv[:, 0:gn], op0=OP.mult, op1=OP.mult)
                    crelu = wpool.tile([128, GW * 128], f32, tag="crelu",
                                       bufs=wb)
                    nc.vector.tensor_scalar(
                        out=crelu[:, 0:gn], in0=psum_cT[:, 0:gn],
                        scalar1=bproj_c, scalar2=0.0, op0=OP.add,
                        op1=OP.max)
                    ctx2 = wpool.tile([128, GW * 128], bf16, tag="ctx2")
                    nc.vector.tensor_tensor(
                        out=ctx2[:, 0:gn], in0=eneg[:, 0:gn],
                        in1=crelu[:, 0:gn], op=OP.add)

                    nfTg = nfT_sb[:, g0 * 128:(g0 + ng) * 128]
                    gb = 2 if GW <= 2 else 1
                    psum_rz = pn_pool.tile([128, GW * 256], f32,
                                           tag="psum_rz", bufs=gb)
                    psum_nh = pn_pool.tile([128, GW * 256], f32,
                                           tag="psum_nh", bufs=gb)
                    nc.tensor.matmul(psum_rz[:, 0:gn],
                                     lhsT=wih_sb[:, 0:128],
                                     rhs=ctx2[:, 0:gn],
                                     start=True, stop=False)
                    nc.tensor.matmul(psum_rz[:, 0:gn],
                                     lhsT=whh_sb[:, 0:128], rhs=nfTg,
                                     start=False, stop=True)
                    nc.tensor.matmul(psum_rz[:, GWn:GWn + gn],
                                     lhsT=wih_sb[:, 128:256],
                                     rhs=ctx2[:, 0:gn],
                                     start=True, stop=False)
                    nc.tensor.matmul(psum_rz[:, GWn:GWn + gn],
                                     lhsT=whh_sb[:, 128:256], rhs=nfTg,
                                     start=False, stop=True)
                    nc.tensor.matmul(psum_nh[:, 0:gn],
                                     lhsT=wih_sb[:, 256:384],
                                     rhs=ctx2[:, 0:gn],
                                     start=True, stop=True)
                    nc.tensor.matmul(psum_nh[:, GWn:GWn + gn],
                                     lhsT=whh_sb[:, 256:384], rhs=nfTg,
                                     start=True, stop=True)

                    sig_r = wpool.tile([128, GW * 128], f32, tag="sig_r",
                                       bufs=wb)
                    nc.scalar.activation(sig_r[:, 0:gn], psum_rz[:, 0:gn],
                                         AF.Sigmoid, bias=br_c)
                    sig_z = wpool.tile([128, GW * 128], bf16, tag="sig_z")
                    nc.scalar.activation(sig_z[:, 0:gn],
                                         psum_rz[:, GWn:GWn + gn],
                                         AF.Sigmoid, bias=bz_c)
                    hnr = wpool.tile([128, GW * 128], f32, tag="hnr",
                                     bufs=wb)
                    nc.vector.scalar_tensor_tensor(
                        out=hnr[:, 0:gn], in0=psum_nh[:, GWn:GWn + gn],
                        scalar=bnh_c, in1=sig_r[:, 0:gn],
                        op0=OP.add, op1=OP.mult)
                    npre = wpool.tile([128, GW * 128], f32, tag="npre",
                                      bufs=wb)
                    nc.vector.tensor_tensor(
                        out=npre[:, 0:gn], in0=hnr[:, 0:gn],
                        in1=psum_nh[:, 0:gn], op=OP.add)
                    nn = wpool.tile([128, GW * 128], bf16, tag="nn")
                    nc.scalar.activation(nn[:, 0:gn], npre[:, 0:gn],
                                         AF.Tanh, bias=bni_c)
                    df = wpool.tile([128, GW * 128], bf16, tag="df")
                    nc.vector.tensor_tensor(
                        out=df[:, 0:gn], in0=nfTg, in1=nn[:, 0:gn],
                        op=OP.subtract)
                    dz = wpool.tile([128, GW * 128], bf16, tag="dz")
                    nc.vector.tensor_tensor(
                        out=dz[:, 0:gn], in0=df[:, 0:gn],
                        in1=sig_z[:, 0:gn], op=OP.mult)
                    hh = wpool.tile([128, GW * 128], bf16, tag="hh")
                    nc.vector.tensor_tensor(
                        out=hh[:, 0:gn], in0=dz[:, 0:gn],
                        in1=nn[:, 0:gn], op=OP.add)
                    outg = wpool.tile([128, GW * 128], f32, tag="outg",
                                      bufs=wb)
                    nc.vector.tensor_scalar(
                        out=outg[:, 0:gn], in0=hh[:, 0:gn], scalar1=0.0,
                        scalar2=None, op0=OP.max)
                    nc.sync.dma_start(
                        out_d[:, g0 * 128:(g0 + ng) * 128],
                        outg[:, 0:gn])

            for _rep in range(repeat):
              ctx_all = xpool.tile([128, WPC * 128], bf16, tag="ctx_all")
              next_g = 0
              # ---- pass 1: gather + edge matmuls + ctx ----
              for b in range(n_batches):
                w0 = b * WPBv
                if il and not skip_node and probe is None:
                    while next_g + GW <= w0:
                        node_group(next_g, ctx_all)
                        next_g += GW
                nw = min(WPBv, WPC - w0)
                a0, a1 = offA[w0], offA[w0 + nw]
                b0, b1 = offB[w0], offB[w0 + nw]
                bA, bB = a1 - a0, b1 - b0
                if skip_gather:
                    GA, GB = GA_static, GB_static
                else:
                    GA = gpool.tile([128, maxbA, 128], bf16, tag="GA",
                                    bufs=gbufs)
                    GB = gpool.tile([128, maxbB, 128], bf16, tag="GB",
                                    bufs=gbufs)
                    if streami:
                        ia_t = gpool.tile([128, maxbA * 8], i16,
                                          tag="ia_t", bufs=gbufs)
                        nc.sync.dma_start(ia_t[:, 0:bA * 8],
                                          idxa_d[:, a0 * 8:a1 * 8])
                        ib_t = gpool.tile([128, maxbB * 8], i16,
                                          tag="ib_t", bufs=gbufs)
                        nc.sync.dma_start(ib_t[:, 0:bB * 8],
                                          idxb_d[:, b0 * 8:b1 * 8])
                        ia_sb, ib_sb = ia_t, ib_t
                        ta0, tb0 = 0, 0
                    else:
                        ia_sb, ib_sb = idxa_sb, idxb_sb
                        ta0, tb0 = a0, b0

                    def qgather(G, tab, idx_sb, t0, nt, qlist, sizes=None):
                        np_ = len(qlist)
                        if sizes is None:
                            per = (nt + np_ - 1) // np_
                            sizes = [per] * np_
                        o = 0
                        for i in range(np_):
                            cn = min(sizes[i], nt - o)
                            if cn <= 0:
                                break
                            ni = cn * 128
                            nc.gpsimd.dma_gather(
                                out_ap=G[:, o:o + cn, :],
                                in_ap=tab,
                                idxs_ap=idx_sb[:, ((t0 + o) * 128) // 16:
                                               ((t0 + o + cn) * 128) // 16],
                                num_idxs=ni, num_idxs_reg=ni, elem_size=128,
                                single_packet=False,
                                queue_num=qlist[i] % max(1, n_q),
                            )
                            o += cn

                    sa_sz = sb_sz = None
                    if qmode == "swap" and (b % 2) == 1:
                        qa, qb_ = (2, 3), (0, 1)
                    elif qmode == "quad":
                        qa, qb_ = (0, 1, 2, 3), (3, 2, 1, 0)
                    elif qmode == "bal":
                        # equalize all 4 queues: each carries (bA+bB)/4
                        qa, qb_ = (0, 1, 2), (2, 3)
                        q_tot = (bA + bB + 3) // 4
                        sa_sz = [q_tot, q_tot, max(0, bA - 2 * q_tot)]
                        sb_sz = [max(0, bB - q_tot), q_tot]
                    else:
                        qa, qb_ = (0, 1), (2, 3)
                    qgather(GA, tabA, ia_sb, ta0, bA, qa, sa_sz)
                    qgather(GB, tabB, ib_sb, tb0, bB, qb_, sb_sz)
                OA = opool.tile([128, maxbA, 128], bf16, tag="OA")
                OB = opool.tile([128, maxbB, 128], bf16, tag="OB")
                GsA = gpool.tile([128, maxbA, 132], bf16, tag="GsA")
                GsB = gpool.tile([128, maxbB, 132], bf16, tag="GsB")
                if not skip_onehot:
                    for (O, dstl_sb, nt, c0) in (
                            (OA, dstla_sb, bA, a0),
                            (OB, dstlb_sb, bB, b0)):
                        nc.vector.tensor_tensor(
                            out=O[:, 0:nt, :],
                            in0=apx(iota_sb[:], [[0, nt], [1, 128]]),
                            in1=apx(dstl_sb[:, c0:c0 + nt],
                                    [[1, nt], [0, 128]]),
                            op=OP.is_equal)
                if probe != "gather":
                    for (G, Gs, ex_sb, nt, c0) in (
                            (GA, GsA, exa_sb, bA, a0),
                            (GB, GsB, exb_sb, bB, b0)):
                        nc.vector.tensor_tensor(
                            out=Gs[:, 0:nt, 0:128], in0=G[:, 0:nt, :],
                            in1=apx(ex_sb[:, c0:c0 + nt],
                                    [[1, nt], [0, 128]]),
                            op=OP.mult)
                        nc.vector.tensor_copy(out=Gs[:, 0:nt, 128:129],
                                              in_=ex_sb[:, c0:c0 + nt])

                if probe is not None:
                    # anti-DCE: give every gather/build a live consumer
                    pr = wpool.tile([128, 4], f32, tag="probe")
                    if probe == "gather":
                        srcs = [GA[:, 0, 0:1], GA[:, (bA + 1) // 2, 0:1],
                                GB[:, 0, 0:1], GB[:, (bB + 1) // 2, 0:1]]
                    else:
                        srcs = [GsA[:, 0, 0:1], GsB[:, 0, 0:1],
                                OA[:, 0, 0:1], OB[:, 0, 0:1]]
                    for i, s in enumerate(srcs):
                        nc.vector.tensor_copy(out=pr[:, i:i + 1], in_=s)
                    nc.sync.dma_start(out_d[:, b * 4:b * 4 + 4], pr[:])
                    continue

                for wl in range(nw):
                    j = w0 + wl
                    la = offA[j] - a0
                    lb = offB[j] - b0
                    sAj, sBj = sAl[j], sBl[j]
                    SWj = sAj + sBj
                    psum_ud = pe_pool.tile([128, 132], f32,
                                           tag="psum_ud", bufs=2)
                    if not skip_mm:
                        for s_ in range(SWj):
                            if s_ < sAj:
                                Olh = OA[:, la + s_, :]
                                Grh = GsA[:, la + s_, 0:129]
                            else:
                                Olh = OB[:, lb + (s_ - sAj), :]
                                Grh = GsB[:, lb + (s_ - sAj), 0:129]
                            nc.tensor.matmul(
                                psum_ud[:, 0:129], lhsT=Olh, rhs=Grh,
                                start=(s_ == 0), stop=(s_ == SWj - 1),
                            )
                    den = wpool.tile([128, 1], f32, tag="den")
                    nc.vector.tensor_scalar(
                        out=den[:], in0=psum_ud[:, 128:129],
                        scalar1=1e-30, scalar2=None, op0=OP.max)
                    rec = wpool.tile([128, 1], f32, tag="rec")
                    nc.vector.reciprocal(rec[:], den[:])
                    nc.vector.tensor_scalar(
                        out=ctx_all[:, j * 128:(j + 1) * 128],
                        in0=psum_ud[:, 0:128],
                        scalar1=rec[:, 0:1], scalar2=None, op0=OP.mult)

              # ---- pass 2: node phase over groups of GW positions ----
              if skip_node or probe is not None:
                  continue
              g0 = next_g if il else 0
              while g0 < WPC:
                  node_group(g0, ctx_all)
                  g0 += GW

    nc.compile()
    return nc


def _prep_v3(edge_logits, node_feats, W_proj, b_proj, w_ih, w_hh, b_ih,
             b_hh, src, dst):
    """Host prep for v3: bin-packed window->position assignment."""
    import ml_dtypes
    bf = ml_dtypes.bfloat16
    logits = np.asarray(edge_logits, np.float32).reshape(-1)
    src = np.asarray(src, np.int64)
    dst = np.asarray(dst, np.int64)

    win = dst // 128
    is_b = (src >= S_SPLIT).astype(np.int64)
    cA = np.bincount(win[is_b == 0], minlength=WTOT)
    cB = np.bincount(win[is_b == 1], minlength=WTOT)

    order = np.argsort(cA, kind="stable")[::-1]   # rank -> window
    rank = np.empty(WTOT, np.int64)
    rank[order] = np.arange(WTOT)
    # position j, core k <- window order[j*NC + k]
    posw = rank // NC      # window -> position
    corew = rank % NC      # window -> core
    wA = cA[order].reshape(WPC, NC)
    wB = cB[order].reshape(WPC, NC)
    sAl = tuple(int(x) for x in np.ceil(wA.max(1) / 128).astype(int))
    sBl = tuple(int(x) for x in np.ceil(wB.max(1) / 128).astype(int))
    offA = np.zeros(WPC + 1, np.int64)
    offA[1:] = np.cumsum(sAl)
    offB = np.zeros(WPC + 1, np.int64)
    offB[1:] = np.cumsum(sBl)
    LAc = int(offA[-1]) * 128
    LBc = int(offB[-1]) * 128

    key = win * 2 + is_b
    order_e = np.argsort(key, kind="stable")
    key_s = key[order_e]
    src_s = src[order_e]
    dst_s = dst[order_e]
    log_s = logits[order_e]
    counts = np.bincount(key_s, minlength=WTOT * 2)
    starts = np.zeros(WTOT * 2, np.int64)
    starts[1:] = np.cumsum(counts)[:-1]
    pos = np.arange(E, dtype=np.int64) - starts[key_s]

    winv = key_s // 2
    grp = key_s % 2
    kv = corew[winv]
    jv = posw[winv]

    idxA = np.zeros(NC * LAc, np.int16)
    idxB = np.zeros(NC * LBc, np.int16)
    dstlA = np.full(NC * LAc, -1.0, np.float32)
    dstlB = np.full(NC * LBc, -1.0, np.float32)
    logA = np.zeros(NC * LAc, np.float32)
    logB = np.zeros(NC * LBc, np.float32)

    mA = grp == 0
    mB = ~mA
    flatA = kv[mA] * LAc + offA[jv[mA]] * 128 + pos[mA]
    flatB = kv[mB] * LBc + offB[jv[mB]] * 128 + pos[mB]
    idxA[flatA] = src_s[mA].astype(np.int16)
    idxB[flatB] = (src_s[mB] - OFF_B).astype(np.int16)
    dstlA[flatA] = (dst_s[mA] - winv[mA] * 128).astype(np.float32)
    dstlB[flatB] = (dst_s[mB] - winv[mB] * 128).astype(np.float32)
    logA[flatA] = log_s[mA]
    logB[flatB] = log_s[mB]

    def core_tiles(a, L, dt):
        a = a.reshape(NC, L // 128, 128)
        return [np.ascontiguousarray(a[k].T.astype(dt)) for k in range(NC)]

    dstlA_cores = core_tiles(dstlA, LAc, bf)
    dstlB_cores = core_tiles(dstlB, LBc, bf)
    logA_cores = core_tiles(logA, LAc, np.float32)
    logB_cores = core_tiles(logB, LBc, np.float32)

    def core_idx(a, L):
        a = a.reshape(NC, L)
        out = []
        for k in range(NC):
            blk = a[k].reshape(-1, 16).T
            out.append(np.ascontiguousarray(np.tile(blk, (8, 1))))
        return out

    idxA_cores = core_idx(idxA, LAc)
    idxB_cores = core_idx(idxB, LBc)

    nf = np.asarray(node_feats, np.float32)
    nf_pad = np.zeros((WTOT * 128, F), np.float32)
    nf_pad[:V] = nf

    table = np.ascontiguousarray(nf.astype(bf))
    tableb = np.ascontiguousarray(table[OFF_B:])
    wprojT = np.ascontiguousarray(np.asarray(W_proj, np.float32).T.astype(bf))
    wihT = np.ascontiguousarray(np.asarray(w_ih, np.float32).T.astype(bf))
    whhT = np.ascontiguousarray(np.asarray(w_hh, np.float32).T.astype(bf))
    bih = np.asarray(b_ih, np.float32).reshape(384)
    bhh = np.asarray(b_hh, np.float32).reshape(384)
    bcols = np.ascontiguousarray(np.stack([
        np.asarray(b_proj, np.float32).reshape(128),
        (bih + bhh)[0:128], (bih + bhh)[128:256],
        bih[256:384], bhh[256:384]], axis=1))
    iota = np.tile(np.arange(128, dtype=np.float32), (128, 1)).astype(bf)
    ident = np.eye(128, dtype=np.float32).astype(bf)

    ow = order.reshape(WPC, NC)   # [position, core] -> window
    in_maps = []
    for k in range(NC):
        widx = ow[:, k]
        node_ids = (widx[:, None] * 128 +
                    np.arange(128)[None, :]).reshape(-1)
        nfT = np.ascontiguousarray(nf_pad[node_ids].T.astype(bf))
        in_maps.append({
            "idxa": idxA_cores[k], "idxb": idxB_cores[k],
            "dstla": dstlA_cores[k], "dstlb": dstlB_cores[k],
            "logita": logA_cores[k], "logitb": logB_cores[k],
            "table": table, "tableb": tableb,
            "nfT": nfT,
            "wprojT": wprojT, "wihT": wihT, "whhT": whhT,
            "bcols": bcols, "iota": iota, "ident": ident,
        })
    return sAl, sBl, order, in_maps


def _unshard_v3(results, order):
    """results: list of per-core 'out' arrays [128, NPC]."""
    big = np.stack(results)                       # [NC, 128, NPC]
    big = big.reshape(NC, 128, WPC, 128)
    # [WPC(pos), NC(core), 128(node), 128(feat)] ordered by rank
    by_rank = big.transpose(2, 0, 3, 1).reshape(WTOT, 128, F)
    out = np.empty((WTOT, 128, F), np.float32)
    out[order.reshape(WPC, NC).reshape(-1)] = by_rank
    return out.reshape(WTOT * 128, F)[:V]


def _prep_v2(edge_logits, node_feats, W_proj, b_proj, w_ih, w_hh, b_ih,
             b_hh, src, dst):
    """Host-side sharding for the v2 kernel (bf16, f-major out)."""
    import ml_dtypes
    bf = ml_dtypes.bfloat16
    logits = np.asarray(edge_logits, np.float32).reshape(-1)
    src = np.asarray(src, np.int64)
    dst = np.asarray(dst, np.int64)

    is_b = (src >= S_SPLIT).astype(np.int64)
    win = dst // 128
    key = win * 2 + is_b
    order = np.argsort(key, kind="stable")
    key_s = key[order]
    src_s = src[order]
    dst_s = dst[order]
    log_s = logits[order]

    counts = np.bincount(key_s, minlength=WTOT * 2)
    cA = counts[0::2]
    cB = counts[1::2]
    sA = int((cA.max() + 127) // 128)
    sB = int((cB.max() + 127) // 128)

    starts = np.zeros(WTOT * 2, np.int64)
    starts[1:] = np.cumsum(counts)[:-1]
    pos = np.arange(E, dtype=np.int64) - starts[key_s]

    winv = key_s // 2
    grp = key_s % 2
    idxA = np.zeros(WTOT * sA * 128, np.int16)
    idxB = np.zeros(WTOT * sB * 128, np.int16)
    dstlA = np.full(WTOT * sA * 128, -1.0, np.float32)
    dstlB = np.full(WTOT * sB * 128, -1.0, np.float32)
    logA = np.zeros(WTOT * sA * 128, np.float32)
    logB = np.zeros(WTOT * sB * 128, np.float32)

    mA = grp == 0
    mB = ~mA
    flatA = winv[mA] * (sA * 128) + pos[mA]
    flatB = winv[mB] * (sB * 128) + pos[mB]
    idxA[flatA] = src_s[mA].astype(np.int16)
    idxB[flatB] = (src_s[mB] - OFF_B).astype(np.int16)
    dstlA[flatA] = (dst_s[mA] - winv[mA] * 128).astype(np.float32)
    dstlB[flatB] = (dst_s[mB] - winv[mB] * 128).astype(np.float32)
    logA[flatA] = log_s[mA]
    logB[flatB] = log_s[mB]

    def core_tiles(a, slots, dt):
        a = a.reshape(WTOT, slots, 128)
        return [np.ascontiguousarray(
            a[k * WPC:(k + 1) * WPC].transpose(2, 0, 1)
            .reshape(128, WPC * slots).astype(dt)) for k in range(NC)]

    dstlA_cores = core_tiles(dstlA, sA, bf)
    dstlB_cores = core_tiles(dstlB, sB, bf)
    logA_cores = core_tiles(logA, sA, np.float32)
    logB_cores = core_tiles(logB, sB, np.float32)

    def core_idx(a, slots):
        a = a.reshape(WTOT, slots * 128)
        out = []
        for k in range(NC):
            flat = a[k * WPC:(k + 1) * WPC].reshape(-1)
            blk = flat.reshape(-1, 16).T
            out.append(np.ascontiguousarray(np.tile(blk, (8, 1))))
        return out

    idxA_cores = core_idx(idxA, sA)
    idxB_cores = core_idx(idxB, sB)

    nf = np.asarray(node_feats, np.float32)
    nf_pad = np.zeros((NC * NPC, F), np.float32)
    nf_pad[:V] = nf

    table = np.ascontiguousarray(nf.astype(bf))
    tableb = np.ascontiguousarray(table[OFF_B:])
    wprojT = np.ascontiguousarray(np.asarray(W_proj, np.float32).T.astype(bf))
    wihT = np.ascontiguousarray(np.asarray(w_ih, np.float32).T.astype(bf))
    whhT = np.ascontiguousarray(np.asarray(w_hh, np.float32).T.astype(bf))
    bih = np.asarray(b_ih, np.float32).reshape(384)
    bhh = np.asarray(b_hh, np.float32).reshape(384)
    bcols = np.stack([
        np.asarray(b_proj, np.float32).reshape(128),
        (bih + bhh)[0:128], (bih + bhh)[128:256],
        bih[256:384], bhh[256:384]], axis=1)
    bcols = np.ascontiguousarray(bcols)
    iota = np.tile(np.arange(128, dtype=np.float32), (128, 1)).astype(bf)
    ident = np.eye(128, dtype=np.float32).astype(bf)

    in_maps = []
    for k in range(NC):
        sl = nf_pad[k * NPC:(k + 1) * NPC]
        nfT = np.ascontiguousarray(sl.T.astype(bf))
        in_maps.append({
            "idxa": idxA_cores[k], "idxb": idxB_cores[k],
            "dstla": dstlA_cores[k], "dstlb": dstlB_cores[k],
            "logita": logA_cores[k], "logitb": logB_cores[k],
            "table": table, "tableb": tableb,
            "nfT": nfT,
            "wprojT": wprojT, "wihT": wihT, "whhT": whhT,
            "bcols": bcols, "iota": iota, "ident": ident,
        })
    return sA, sB, in_maps


def kernel(edge_logits, node_feats, W_proj, b_proj, w_ih, w_hh, b_ih, b_hh,
           src, dst):
    from concourse.bass_utils import run_bass_kernel_spmd

    sAl, sBl, order, in_maps = _prep_v3(edge_logits, node_feats, W_proj,
                                        b_proj, w_ih, w_hh, b_ih, b_hh,
                                        src, dst)
    key = ("v3", sAl, sBl)
    if key not in _compiled:
        _compiled[key] = _build_v3(sAl, sBl, WPBv=3, gbufs=3, GW=4, wb=1)
    nc = _compiled[key]

    res = run_bass_kernel_spmd(nc, in_maps, list(range(NC)))
    out = _unshard_v3([res.results[k]["out"] for k in range(NC)], order)
    return np.ascontiguousarray(out).astype(np.float32)

